# revision 19
# baseline (speedup 1.0000x reference)
"""BipartiteSAGEConv Trainium2 kernel.

Strategy: destination-sharded, zero collectives.
- Host: sort/partition edges by destination across 8 cores (6250 dsts each),
  group per 128-dst tile, split by src half (int16 index limit), pad to
  128-edge chunks (uniform chunk structure across cores so one SPMD program
  serves all 8 cores). Pad indices are -1 (skipped by gather ucode).
  Per-dst 1/deg is precomputed on host and uploaded (no count matmuls).
- Device per core: dma_gather (MoE row-gather ucode) pulls per-edge src rows
  HBM->SBUF in f16; scatter-add via one-hot matmul (f16) on the TensorEngine
  accumulates [dst,128] sums in PSUM; scale by 1/deg; transpose; two linear
  layers + bias via PE matmuls (all f16 operands, f32 PSUM); DMA out the
  [6250,128] f32 shard.
"""

import sys
import types

import numpy as np

N_SRC = 50000
N_DST = 50000
E = 800000
D = 128
OUT = 128
N_CORES = 8
P = 128
DST_PER_CORE = N_DST // N_CORES          # 6250
TILES = (DST_PER_CORE + P - 1) // P      # 49
HALF = 25000                             # int16 index limit split
# SWDGE ring limit: 1024 rows/gather (1920 wedges the device: NRT 101).
MAX_ROWS_PER_GATHER = 1024


def _install_ntff_hook():
    try:
        import antenv
        if "antenv.axon_hooks" in sys.modules:
            return
        mod = types.ModuleType("antenv.axon_hooks")
        _h = [None]
        mod.set_axon_ntff_profile_hook = lambda h: _h.__setitem__(0, h)
        mod.get_axon_ntff_profile_hook = lambda: _h[0]
        sys.modules["antenv.axon_hooks"] = mod
        antenv.axon_hooks = mod
        from trn_agent_boot.trn_boot import _ntff_profile_via_ctypes
        mod.set_axon_ntff_profile_hook(
            _ntff_profile_via_ctypes("/opt/axon/libaxon_pjrt.so"))
    except Exception:
        pass


def _balance_dsts(edge_src, edge_dst):
    """Assign each dst to a (core, tile, slot) bin so that per-(bin, src-half)
    edge counts are balanced (target <=1024 = 8 chunks of 128).

    Returns (dst_core, dst_tile, dst_slot, bins) where bins[c][t] is the
    ordered list of dst ids in that bin.
    """
    n_bins = N_CORES * TILES                     # 392
    lo_deg = np.bincount(edge_dst[edge_src < HALF], minlength=N_DST)
    hi_deg = np.bincount(edge_dst[edge_src >= HALF], minlength=N_DST)
    # bin capacities: last tile of each core holds the 6250-48*128=106 rest
    cap = np.full(n_bins, P, np.int64)
    cap[TILES - 1::TILES] = DST_PER_CORE - (TILES - 1) * P   # 106
    order = np.argsort(-(lo_deg + hi_deg), kind="stable")
    bin_lo = np.zeros(n_bins, np.int64)
    bin_hi = np.zeros(n_bins, np.int64)
    bin_cnt = np.zeros(n_bins, np.int64)
    members = [[] for _ in range(n_bins)]
    full = np.zeros(n_bins, bool)
    for d in order:
        load = np.maximum(bin_lo + lo_deg[d], bin_hi + hi_deg[d]).astype(
            np.float64)
        load[full] = np.inf
        b = int(np.argmin(load))
        members[b].append(int(d))
        bin_lo[b] += lo_deg[d]
        bin_hi[b] += hi_deg[d]
        bin_cnt[b] += 1
        if bin_cnt[b] >= cap[b]:
            full[b] = True
    dst_core = np.empty(N_DST, np.int64)
    dst_tile = np.empty(N_DST, np.int64)
    dst_slot = np.empty(N_DST, np.int64)
    bins = [[None] * TILES for _ in range(N_CORES)]
    for b in range(n_bins):
        c, t = divmod(b, TILES)
        ids = np.array(members[b], np.int64)
        bins[c][t] = ids
        dst_core[ids] = c
        dst_tile[ids] = t
        dst_slot[ids] = np.arange(len(ids))
    return dst_core, dst_tile, dst_slot, bins


def _prep_core(edge_src, edge_dst, core, dst_core, dst_tile, dst_slot):
    """Per-core edge structure: for each (tile, half) return the edge lists.

    Returns list over 49 tiles of (src_lo, dstl_lo, src_hi, dstl_hi) where
    src_* are int64 source indices (absolute) and dstl_* are slot-in-tile ids.
    """
    m = dst_core[edge_dst] == core
    es = edge_src[m]
    tid = dst_tile[edge_dst[m]]
    dl = dst_slot[edge_dst[m]]
    order = np.argsort(tid, kind="stable")
    es, tid, dl = es[order], tid[order], dl[order]
    bounds = np.searchsorted(tid, np.arange(TILES + 1))
    tiles = []
    for t in range(TILES):
        a, b = bounds[t], bounds[t + 1]
        s, d = es[a:b], dl[a:b]
        is_lo = s < HALF
        tiles.append((s[is_lo], d[is_lo], s[~is_lo] - HALF, d[~is_lo]))
    return tiles


def _pad_chunks(src, dstl, n_chunks):
    """Pad to n_chunks*128 edges; pad idx=0 (valid row), dstl=-1 (no one-hot)."""
    n = n_chunks * P
    s = np.zeros(n, np.int16)
    d = np.full(n, -1.0, np.float32)
    s[:len(src)] = src.astype(np.int16)
    d[:len(dstl)] = dstl.astype(np.float32)
    return s, d


def _wrap_idx(idx_flat):
    """dma_gather wrapped index layout: index j at partition j%16, col j//16,
    replicated across the 8 gpsimd cores (partition groups of 16)."""
    n = len(idx_flat)
    w = idx_flat.reshape(n // 16, 16).T          # [16, n/16]
    return np.tile(w, (8, 1))                    # [128, n/16]


def build_and_run(x_src, x_dst, edge_src, edge_dst, W_neigh, b_neigh,
                  W_self, b_self):
    _install_ntff_hook()
    from concourse import bacc, bass, mybir, tile
    from concourse.bass_utils import run_bass_kernel_spmd
    from concourse.masks import make_identity

    F32 = mybir.dt.float32
    F16 = mybir.dt.float16
    import os as _os
    use_f16 = _os.environ.get("BSAGE_F32", "0") != "1"
    DTAB = F16 if use_f16 else F32
    np_tab = np.float16 if use_f16 else np.float32

    # ---------- host-side sharding / layout ----------
    dst_core, dst_tile, dst_slot, bins = _balance_dsts(edge_src, edge_dst)
    per_core_tiles = [
        _prep_core(edge_src, edge_dst, c, dst_core, dst_tile, dst_slot)
        for c in range(N_CORES)]
    # dst ids of core c in output-row order
    core_dst_ids = [np.concatenate(bins[c]) for c in range(N_CORES)]

    # per-dst reciprocal degree, tile-major per core: rdeg[c][p, t]
    deg = np.bincount(edge_dst, minlength=N_DST).astype(np.float32)
    rdeg_full = 1.0 / np.maximum(deg, 1.0)
    rdeg_all = np.zeros((N_CORES, P, TILES), np.float32)
    for c in range(N_CORES):
        shard = np.zeros(TILES * P, np.float32)
        for t in range(TILES):
            ids = bins[c][t]
            shard[t * P:t * P + len(ids)] = rdeg_full[ids]
        rdeg_all[c] = shard.reshape(TILES, P).T

    # uniform chunk counts across cores (SPMD: one program, 8 data sets)
    KL = [max(max(1, -(-len(per_core_tiles[c][t][0]) // P))
              for c in range(N_CORES)) for t in range(TILES)]
    KH = [max(max(1, -(-len(per_core_tiles[c][t][2]) // P))
              for c in range(N_CORES)) for t in range(TILES)]
    KE = [KL[t] + KH[t] for t in range(TILES)]
    NCH = sum(KE)                                 # total chunks per core
    KEMAX = max(KE)

    # quad grouping: a few tiles share one g buffer; chunk layout within a
    # quad: [lo(t0)|lo(t1)|...|hi(t0)|hi(t1)|...]
    _sizes = [8] * ((TILES - 9) // 8) + [4, 2, 2, 1]
    _rem = TILES - sum(_sizes)
    _sizes = [8] * (_rem // 8) + ([_rem % 8] if _rem % 8 else []) + _sizes if _rem > 0 else _sizes
    QUADS = []
    _q = 0
    for _s in _sizes:
        QUADS.append(list(range(_q, _q + _s)))
        _q += _s
    assert _q == TILES, (_q, TILES, _sizes)
    # chunk offset of each (tile, half) within its quad buffer
    chunk_off = {}
    quad_chunks = []
    for qi, qts in enumerate(QUADS):
        off = 0
        for t in qts:
            chunk_off[(t, 0)] = off
            off += KL[t]
        for t in qts:
            chunk_off[(t, 1)] = off
            off += KH[t]
        quad_chunks.append(off)
    KQMAX = max(quad_chunks)

    # gather plan: per quad per half, one contiguous chunk span covering the
    # member tiles' chunks, split into <=8-chunk (1024-row) instructions.
    # gathers: (quad, half, chunk_off_in_quad, n_chunks, idx_col_base)
    gathers = []
    idx_cols = 0                                  # int16 columns consumed
    for qi, qts in enumerate(QUADS):
        for half in (0, 1):
            span = sum((KL if half == 0 else KH)[t] for t in qts)
            base = chunk_off[(qts[0], half)]
            k_done = 0
            while k_done < span:
                k = min(span - k_done, MAX_ROWS_PER_GATHER // P)
                gathers.append((qi, half, base + k_done, k, idx_cols))
                idx_cols += k * 8
                k_done += k
    IDXCOLS = idx_cols

    # per-core data arrays
    idx_all = np.zeros((N_CORES, P, IDXCOLS), np.int16)
    dstl_all = np.zeros((N_CORES, P, NCH), np.float16)
    cbase = np.concatenate([[0], np.cumsum(KE)])  # chunk col base per tile
    for c in range(N_CORES):
        for t in range(TILES):
            s_lo, d_lo, s_hi, d_hi = per_core_tiles[c][t]
            sl, dl = _pad_chunks(s_lo, d_lo, KL[t])
            sh, dh = _pad_chunks(s_hi, d_hi, KH[t])
            d_cat = np.concatenate([dl, dh])
            # dstl layout: [128, NCH]; slot p of chunk k = edge k*128+p
            dstl_all[c][:, cbase[t]:cbase[t + 1]] = (
                d_cat.reshape(KE[t], P).T.astype(np.float16))
        # per-quad padded source-index streams (chunk layout order)
        quad_src = []
        for qi, qts in enumerate(QUADS):
            parts = []
            for t in qts:
                s_lo, d_lo, _, _ = per_core_tiles[c][t]
                parts.append(_pad_chunks(s_lo, d_lo, KL[t])[0])
            for t in qts:
                _, _, s_hi, d_hi = per_core_tiles[c][t]
                parts.append(_pad_chunks(s_hi, d_hi, KH[t])[0])
            quad_src.append(np.concatenate(parts))
        for (qi, half, off, k, colb) in gathers:
            rows = quad_src[qi][off * P:(off + k) * P]
            idx_all[c][:, colb:colb + k * 8] = _wrap_idx(rows)

    x_lo = np.ascontiguousarray(x_src[:HALF]).astype(np_tab)
    x_hi = np.ascontiguousarray(x_src[HALF:]).astype(np_tab)
    xdstT = np.zeros((N_CORES, P, TILES * P), np.float16)
    for c in range(N_CORES):
        for t in range(TILES):
            ids = bins[c][t]
            xdstT[c][:, t * P:t * P + len(ids)] = (
                x_dst[ids].T.astype(np.float16))
    iota = np.tile(np.arange(P, dtype=np.float16), (P, 1))
    wn = W_neigh.astype(np.float16)
    ws = W_self.astype(np.float16)
    bsum = (b_neigh + b_self).astype(np.float16)[None, :]  # [1,128]

    # ---------- device program ----------
    nc = bacc.Bacc("TRN2", target_bir_lowering=False, debug=False,
                   num_devices=N_CORES, num_swdge_queues=4)
    xlo_d = nc.dram_tensor("xlo", [HALF, D], DTAB, kind="ExternalInput").ap()
    xhi_d = nc.dram_tensor("xhi", [HALF, D], DTAB, kind="ExternalInput").ap()
    idx_d = nc.dram_tensor("idx", [P, IDXCOLS], mybir.dt.int16,
                           kind="ExternalInput").ap()
    dstl_d = nc.dram_tensor("dstl", [P, NCH], F16, kind="ExternalInput").ap()
    rdeg_d = nc.dram_tensor("rdeg", [P, TILES], F32, kind="ExternalInput").ap()
    xdstT_d = nc.dram_tensor("xdstT", [P, TILES * P], F16,
                             kind="ExternalInput").ap()
    iota_d = nc.dram_tensor("iota", [P, P], F16, kind="ExternalInput").ap()
    wn_d = nc.dram_tensor("wn", [D, OUT], F16, kind="ExternalInput").ap()
    ws_d = nc.dram_tensor("ws", [D, OUT], F16, kind="ExternalInput").ap()
    bsum_d = nc.dram_tensor("bsum", [1, OUT], F16, kind="ExternalInput").ap()
    out_d = nc.dram_tensor("out", [DST_PER_CORE, OUT], F16,
                           kind="ExternalOutput").ap()

    with tile.TileContext(nc) as tc:
        with (
            tc.tile_pool(name="const", bufs=1) as cpool,
            tc.tile_pool(name="work", bufs=4) as wpool,
            tc.tile_pool(name="psum", bufs=2, space="PSUM") as ppool,
        ):
            idx_sb = cpool.tile([P, IDXCOLS], mybir.dt.int16)
            dstl_sb = cpool.tile([P, NCH], F16)
            rdeg_sb = cpool.tile([P, TILES], F32)
            xdstT_sb = cpool.tile([P, TILES * P], F16)
            iota_sb = cpool.tile([P, P], F16)
            wn_sb = cpool.tile([D, OUT], F16)
            ws_sb = cpool.tile([D, OUT], F16)
            bsum_sb = cpool.tile([1, OUT], F16)
            ones_row = cpool.tile([1, P], F16)
            ident_sb = cpool.tile([P, P], F32)
            cols_g0 = max(g[4] + g[3] * 8 for g in gathers if g[0] == 0)
            nc.sync.dma_start(out=idx_sb[:, :cols_g0], in_=idx_d[:, :cols_g0])
            nc.sync.dma_start(out=idx_sb[:, cols_g0:], in_=idx_d[:, cols_g0:])
            nc.sync.dma_start(out=dstl_sb[:], in_=dstl_d[:])
            nc.sync.dma_start(out=iota_sb[:], in_=iota_d[:])
            nc.scalar.dma_start(out=rdeg_sb[:], in_=rdeg_d[:])
            nc.scalar.dma_start(out=xdstT_sb[:], in_=xdstT_d[:])
            nc.scalar.dma_start(out=wn_sb[:], in_=wn_d[:])
            nc.scalar.dma_start(out=ws_sb[:], in_=ws_d[:])
            nc.scalar.dma_start(out=bsum_sb[:], in_=bsum_d[:])
            nc.vector.memset(ones_row[:], 1.0)
            make_identity(nc, ident_sb[:])

            def _emit_tile(t, g_sb):
                ke = KE[t]
                # batched one-hot: oh[p, k*128+j] = (iota[p,j] == dstl[p,cb+k])
                oh_sb = wpool.tile([P, KEMAX * P], DTAB, tag="oh", name=f"oh{t}")
                i_ap = iota_sb[:]
                iota3d = bass.AP(i_ap.tensor, i_ap.offset,
                                 [i_ap.ap[0], [0, ke], [i_ap.ap[1][0], P]])
                d_ap = dstl_sb[:]
                dstl3d = bass.AP(d_ap.tensor, d_ap.offset + int(cbase[t]),
                                 [d_ap.ap[0], [d_ap.ap[1][0], ke], [0, P]])
                oh3d = bass.AP(oh_sb[:].tensor, oh_sb[:].offset,
                               [oh_sb[:].ap[0], [P, ke], [1, P]])
                nc.vector.tensor_tensor(out=oh3d, in0=iota3d, in1=dstl3d,
                                        op=mybir.AluOpType.is_equal)

                ps1 = ppool.tile([P, D], F32, tag="ps1", name=f"ps1_{t}",
                                 space="PSUM", bufs=3)
                def gchunk(k):
                    if k < KL[t]:
                        return chunk_off[(t, 0)] + k
                    return chunk_off[(t, 1)] + k - KL[t]
                for k in range(ke):
                    gk = gchunk(k)
                    nc.tensor.matmul(
                        out=ps1[:, 0:D],
                        lhsT=oh_sb[:, k * P:(k + 1) * P],
                        rhs=g_sb[:, gk * P:(gk + 1) * P],
                        start=(k == 0), stop=(k == ke - 1))

                # agg = sums * (1/deg): per-partition scale on the scalar eng
                agg_sb = wpool.tile([P, D], F32, tag="agg", name=f"agg{t}")
                nc.scalar.mul(out=agg_sb[:], in_=ps1[:, 0:D],
                              mul=rdeg_sb[:, t:t + 1])
                ps_t = ppool.tile([P, P], F32, tag="pst", name=f"pst{t}",
                                  space="PSUM", bufs=3)
                nc.tensor.transpose(out=ps_t[:], in_=agg_sb[:],
                                    identity=ident_sb[:])
                aggT_sb = wpool.tile([P, D], F16, tag="aggT", name=f"agT{t}")
                nc.vector.tensor_copy(out=aggT_sb[:], in_=ps_t[:])

                ps2 = ppool.tile([P, OUT], F32, tag="ps2", name=f"ps2_{t}",
                                 space="PSUM")
                nc.tensor.matmul(out=ps2[:], lhsT=aggT_sb[:], rhs=wn_sb[:],
                                 start=True, stop=False)
                nc.tensor.matmul(out=ps2[:],
                                 lhsT=xdstT_sb[:, t * P:(t + 1) * P],
                                 rhs=ws_sb[:], start=False, stop=False)
                nc.tensor.matmul(out=ps2[:], lhsT=ones_row[:], rhs=bsum_sb[:],
                                 start=False, stop=True)
                o_sb = wpool.tile([P, OUT], F16, tag="osb", name=f"o{t}")
                nc.scalar.copy(out=o_sb[:], in_=ps2[:])
                rows = min(P, DST_PER_CORE - t * P)
                nc.sync.dma_start(out=out_d[t * P:t * P + rows, :],
                                  in_=o_sb[:rows, :])

            gq = [0]
            g_by_quad = [[] for _ in range(len(QUADS))]
            for g in gathers:
                g_by_quad[g[0]].append(g)
            for qi, qts in enumerate(QUADS):
                g_sb = wpool.tile([P, KQMAX * P], DTAB, tag="g", name=f"g{qi}", bufs=3)
                for (_, half, off, k, colb) in g_by_quad[qi]:
                    t_ap = g_sb[:]
                    out3d = bass.AP(t_ap.tensor, t_ap.offset + off * P,
                                    [t_ap.ap[0], [P, k], [1, P]])
                    nc.gpsimd.dma_gather(
                        out3d,
                        (xlo_d if half == 0 else xhi_d)[:],
                        idx_sb[:, colb:colb + k * 8],
                        k * P,
                        k * P,
                        D,
                        queue_num=(gq[0] % 4),
                    )
                    gq[0] += 1
                for t in qts:
                    _emit_tile(t, g_sb)

    nc.finalize()

    in_maps = [{
        "xlo": x_lo, "xhi": x_hi, "idx": idx_all[c], "dstl": dstl_all[c],
        "rdeg": rdeg_all[c], "xdstT": xdstT[c], "iota": iota, "wn": wn,
        "ws": ws, "bsum": bsum,
    } for c in range(N_CORES)]

    import os
    trace = os.environ.get("BSAGE_TRACE", "0") == "1"
    res = run_bass_kernel_spmd(nc, in_maps, core_ids=list(range(N_CORES)),
                               trace=trace)
    out = np.zeros((N_DST, OUT), np.float32)
    for c in range(N_CORES):
        out[core_dst_ids[c]] = res.results[c]["out"].astype(np.float32)
    if trace:
        build_and_run.last_exec_ns = res.exec_time_ns
    return out


def kernel(x_src, x_dst, edge_src, edge_dst, num_dst, W_neigh, b_neigh,
           W_self, b_self):
    x_src = np.asarray(x_src, dtype=np.float32)
    x_dst = np.asarray(x_dst, dtype=np.float32)
    edge_src = np.asarray(edge_src).astype(np.int64)
    edge_dst = np.asarray(edge_dst).astype(np.int64)
    W_neigh = np.asarray(W_neigh, dtype=np.float32)
    b_neigh = np.asarray(b_neigh, dtype=np.float32)
    W_self = np.asarray(W_self, dtype=np.float32)
    b_self = np.asarray(b_self, dtype=np.float32)
    return build_and_run(x_src, x_dst, edge_src, edge_dst, W_neigh, b_neigh,
                         W_self, b_self)


# revision 20
# speedup vs baseline: 1.0587x; 1.0587x over previous
"""BipartiteSAGEConv Trainium2 kernel.

Strategy: destination-sharded, zero collectives.
- Host: sort/partition edges by destination across 8 cores (6250 dsts each),
  group per 128-dst tile, split by src half (int16 index limit), pad to
  128-edge chunks (uniform chunk structure across cores so one SPMD program
  serves all 8 cores). Pad indices are -1 (skipped by gather ucode).
  Per-dst 1/deg is precomputed on host and uploaded (no count matmuls).
- Device per core: dma_gather (MoE row-gather ucode) pulls per-edge src rows
  HBM->SBUF in f16; scatter-add via one-hot matmul (f16) on the TensorEngine
  accumulates [dst,128] sums in PSUM; scale by 1/deg; transpose; two linear
  layers + bias via PE matmuls (all f16 operands, f32 PSUM); DMA out the
  [6250,128] f32 shard.
"""

import sys
import types

import numpy as np

N_SRC = 50000
N_DST = 50000
E = 800000
D = 128
OUT = 128
N_CORES = 8
P = 128
DST_PER_CORE = N_DST // N_CORES          # 6250
TILES = (DST_PER_CORE + P - 1) // P      # 49
HALF = 25000                             # int16 index limit split
# SWDGE ring limit: 1024 rows/gather (1920 wedges the device: NRT 101).
MAX_ROWS_PER_GATHER = 1024


def _install_ntff_hook():
    try:
        import antenv
        if "antenv.axon_hooks" in sys.modules:
            return
        mod = types.ModuleType("antenv.axon_hooks")
        _h = [None]
        mod.set_axon_ntff_profile_hook = lambda h: _h.__setitem__(0, h)
        mod.get_axon_ntff_profile_hook = lambda: _h[0]
        sys.modules["antenv.axon_hooks"] = mod
        antenv.axon_hooks = mod
        from trn_agent_boot.trn_boot import _ntff_profile_via_ctypes
        mod.set_axon_ntff_profile_hook(
            _ntff_profile_via_ctypes("/opt/axon/libaxon_pjrt.so"))
    except Exception:
        pass


def _balance_dsts(edge_src, edge_dst):
    """Assign each dst to a (core, tile, slot) bin so that per-(bin, src-half)
    edge counts are balanced (target <=1024 = 8 chunks of 128).

    Returns (dst_core, dst_tile, dst_slot, bins) where bins[c][t] is the
    ordered list of dst ids in that bin.
    """
    n_bins = N_CORES * TILES                     # 392
    lo_deg = np.bincount(edge_dst[edge_src < HALF], minlength=N_DST)
    hi_deg = np.bincount(edge_dst[edge_src >= HALF], minlength=N_DST)
    # bin capacities: last tile of each core holds the 6250-48*128=106 rest
    cap = np.full(n_bins, P, np.int64)
    cap[TILES - 1::TILES] = DST_PER_CORE - (TILES - 1) * P   # 106
    order = np.argsort(-(lo_deg + hi_deg), kind="stable")
    bin_lo = np.zeros(n_bins, np.int64)
    bin_hi = np.zeros(n_bins, np.int64)
    bin_cnt = np.zeros(n_bins, np.int64)
    members = [[] for _ in range(n_bins)]
    full = np.zeros(n_bins, bool)
    lim = 8 * P                                  # 1024-edge half target
    for d in order:
        nlo = bin_lo + lo_deg[d]
        nhi = bin_hi + hi_deg[d]
        load = np.maximum(nlo, nhi).astype(np.float64)
        load += 1e6 * (np.maximum(nlo - lim, 0) + np.maximum(nhi - lim, 0))
        load[full] = np.inf
        b = int(np.argmin(load))
        members[b].append(int(d))
        bin_lo[b] += lo_deg[d]
        bin_hi[b] += hi_deg[d]
        bin_cnt[b] += 1
        if bin_cnt[b] >= cap[b]:
            full[b] = True
    dst_core = np.empty(N_DST, np.int64)
    dst_tile = np.empty(N_DST, np.int64)
    dst_slot = np.empty(N_DST, np.int64)
    bins = [[None] * TILES for _ in range(N_CORES)]
    for b in range(n_bins):
        c, t = divmod(b, TILES)
        ids = np.array(members[b], np.int64)
        bins[c][t] = ids
        dst_core[ids] = c
        dst_tile[ids] = t
        dst_slot[ids] = np.arange(len(ids))
    return dst_core, dst_tile, dst_slot, bins


def _prep_core(edge_src, edge_dst, core, dst_core, dst_tile, dst_slot):
    """Per-core edge structure: for each (tile, half) return the edge lists.

    Returns list over 49 tiles of (src_lo, dstl_lo, src_hi, dstl_hi) where
    src_* are int64 source indices (absolute) and dstl_* are slot-in-tile ids.
    """
    m = dst_core[edge_dst] == core
    es = edge_src[m]
    tid = dst_tile[edge_dst[m]]
    dl = dst_slot[edge_dst[m]]
    order = np.argsort(tid, kind="stable")
    es, tid, dl = es[order], tid[order], dl[order]
    bounds = np.searchsorted(tid, np.arange(TILES + 1))
    tiles = []
    for t in range(TILES):
        a, b = bounds[t], bounds[t + 1]
        s, d = es[a:b], dl[a:b]
        is_lo = s < HALF
        tiles.append((s[is_lo], d[is_lo], s[~is_lo] - HALF, d[~is_lo]))
    return tiles


def _pad_chunks(src, dstl, n_chunks):
    """Pad to n_chunks*128 edges; pad idx=0 (valid row), dstl=-1 (no one-hot)."""
    n = n_chunks * P
    s = np.zeros(n, np.int16)
    d = np.full(n, -1.0, np.float32)
    s[:len(src)] = src.astype(np.int16)
    d[:len(dstl)] = dstl.astype(np.float32)
    return s, d


def _wrap_idx(idx_flat):
    """dma_gather wrapped index layout: index j at partition j%16, col j//16,
    replicated across the 8 gpsimd cores (partition groups of 16)."""
    n = len(idx_flat)
    w = idx_flat.reshape(n // 16, 16).T          # [16, n/16]
    return np.tile(w, (8, 1))                    # [128, n/16]


def build_and_run(x_src, x_dst, edge_src, edge_dst, W_neigh, b_neigh,
                  W_self, b_self):
    _install_ntff_hook()
    from concourse import bacc, bass, mybir, tile
    from concourse.bass_utils import run_bass_kernel_spmd
    from concourse.masks import make_identity

    F32 = mybir.dt.float32
    F16 = mybir.dt.float16
    import os as _os
    use_f16 = _os.environ.get("BSAGE_F32", "0") != "1"
    DTAB = F16 if use_f16 else F32
    np_tab = np.float16 if use_f16 else np.float32

    # ---------- host-side sharding / layout ----------
    dst_core, dst_tile, dst_slot, bins = _balance_dsts(edge_src, edge_dst)
    per_core_tiles = [
        _prep_core(edge_src, edge_dst, c, dst_core, dst_tile, dst_slot)
        for c in range(N_CORES)]
    # dst ids of core c in output-row order
    core_dst_ids = [np.concatenate(bins[c]) for c in range(N_CORES)]

    # per-dst reciprocal degree, tile-major per core: rdeg[c][p, t]
    deg = np.bincount(edge_dst, minlength=N_DST).astype(np.float32)
    rdeg_full = 1.0 / np.maximum(deg, 1.0)
    rdeg_all = np.zeros((N_CORES, P, TILES), np.float32)
    for c in range(N_CORES):
        shard = np.zeros(TILES * P, np.float32)
        for t in range(TILES):
            ids = bins[c][t]
            shard[t * P:t * P + len(ids)] = rdeg_full[ids]
        rdeg_all[c] = shard.reshape(TILES, P).T

    # uniform chunk counts across cores (SPMD: one program, 8 data sets)
    KL = [max(max(1, -(-len(per_core_tiles[c][t][0]) // P))
              for c in range(N_CORES)) for t in range(TILES)]
    KH = [max(max(1, -(-len(per_core_tiles[c][t][2]) // P))
              for c in range(N_CORES)) for t in range(TILES)]
    KE = [KL[t] + KH[t] for t in range(TILES)]
    NCH = sum(KE)                                 # total chunks per core
    KEMAX = max(KE)

    # quad grouping: a few tiles share one g buffer; chunk layout within a
    # quad: [lo(t0)|lo(t1)|...|hi(t0)|hi(t1)|...]
    _sizes = [8] * ((TILES - 9) // 8) + [4, 2, 2, 1]
    _rem = TILES - sum(_sizes)
    _sizes = [8] * (_rem // 8) + ([_rem % 8] if _rem % 8 else []) + _sizes if _rem > 0 else _sizes
    QUADS = []
    _q = 0
    for _s in _sizes:
        QUADS.append(list(range(_q, _q + _s)))
        _q += _s
    assert _q == TILES, (_q, TILES, _sizes)
    # chunk offset of each (tile, half) within its quad buffer
    chunk_off = {}
    quad_chunks = []
    for qi, qts in enumerate(QUADS):
        off = 0
        for t in qts:
            chunk_off[(t, 0)] = off
            off += KL[t]
        for t in qts:
            chunk_off[(t, 1)] = off
            off += KH[t]
        quad_chunks.append(off)
    KQMAX = max(quad_chunks)

    # gather plan: per quad per half, one contiguous chunk span covering the
    # member tiles' chunks, split into <=8-chunk (1024-row) instructions.
    # gathers: (quad, half, chunk_off_in_quad, n_chunks, idx_col_base)
    gathers = []
    idx_cols = 0                                  # int16 columns consumed
    for qi, qts in enumerate(QUADS):
        for half in (0, 1):
            span = sum((KL if half == 0 else KH)[t] for t in qts)
            base = chunk_off[(qts[0], half)]
            k_done = 0
            while k_done < span:
                k = min(span - k_done, MAX_ROWS_PER_GATHER // P)
                gathers.append((qi, half, base + k_done, k, idx_cols))
                idx_cols += k * 8
                k_done += k
    IDXCOLS = idx_cols

    # per-core data arrays
    idx_all = np.zeros((N_CORES, P, IDXCOLS), np.int16)
    dstl_all = np.zeros((N_CORES, P, NCH), np.float16)
    cbase = np.concatenate([[0], np.cumsum(KE)])  # chunk col base per tile
    for c in range(N_CORES):
        for t in range(TILES):
            s_lo, d_lo, s_hi, d_hi = per_core_tiles[c][t]
            sl, dl = _pad_chunks(s_lo, d_lo, KL[t])
            sh, dh = _pad_chunks(s_hi, d_hi, KH[t])
            d_cat = np.concatenate([dl, dh])
            # dstl layout: [128, NCH]; slot p of chunk k = edge k*128+p
            dstl_all[c][:, cbase[t]:cbase[t + 1]] = (
                d_cat.reshape(KE[t], P).T.astype(np.float16))
        # per-quad padded source-index streams (chunk layout order)
        quad_src = []
        for qi, qts in enumerate(QUADS):
            parts = []
            for t in qts:
                s_lo, d_lo, _, _ = per_core_tiles[c][t]
                parts.append(_pad_chunks(s_lo, d_lo, KL[t])[0])
            for t in qts:
                _, _, s_hi, d_hi = per_core_tiles[c][t]
                parts.append(_pad_chunks(s_hi, d_hi, KH[t])[0])
            quad_src.append(np.concatenate(parts))
        for (qi, half, off, k, colb) in gathers:
            rows = quad_src[qi][off * P:(off + k) * P]
            idx_all[c][:, colb:colb + k * 8] = _wrap_idx(rows)

    x_lo = np.ascontiguousarray(x_src[:HALF]).astype(np_tab)
    x_hi = np.ascontiguousarray(x_src[HALF:]).astype(np_tab)
    xdstT = np.zeros((N_CORES, P, TILES * P), np.float16)
    for c in range(N_CORES):
        for t in range(TILES):
            ids = bins[c][t]
            xdstT[c][:, t * P:t * P + len(ids)] = (
                x_dst[ids].T.astype(np.float16))
    iota = np.tile(np.arange(P, dtype=np.float16), (P, 1))
    wn = W_neigh.astype(np.float16)
    ws = W_self.astype(np.float16)
    bsum = (b_neigh + b_self).astype(np.float16)[None, :]  # [1,128]

    # ---------- device program ----------
    nc = bacc.Bacc("TRN2", target_bir_lowering=False, debug=False,
                   num_devices=N_CORES, num_swdge_queues=4)
    xlo_d = nc.dram_tensor("xlo", [HALF, D], DTAB, kind="ExternalInput").ap()
    xhi_d = nc.dram_tensor("xhi", [HALF, D], DTAB, kind="ExternalInput").ap()
    idx_d = nc.dram_tensor("idx", [P, IDXCOLS], mybir.dt.int16,
                           kind="ExternalInput").ap()
    dstl_d = nc.dram_tensor("dstl", [P, NCH], F16, kind="ExternalInput").ap()
    rdeg_d = nc.dram_tensor("rdeg", [P, TILES], F32, kind="ExternalInput").ap()
    xdstT_d = nc.dram_tensor("xdstT", [P, TILES * P], F16,
                             kind="ExternalInput").ap()
    iota_d = nc.dram_tensor("iota", [P, P], F16, kind="ExternalInput").ap()
    wn_d = nc.dram_tensor("wn", [D, OUT], F16, kind="ExternalInput").ap()
    ws_d = nc.dram_tensor("ws", [D, OUT], F16, kind="ExternalInput").ap()
    bsum_d = nc.dram_tensor("bsum", [1, OUT], F16, kind="ExternalInput").ap()
    out_d = nc.dram_tensor("out", [DST_PER_CORE, OUT], F16,
                           kind="ExternalOutput").ap()

    with tile.TileContext(nc) as tc:
        with (
            tc.tile_pool(name="const", bufs=1) as cpool,
            tc.tile_pool(name="work", bufs=4) as wpool,
            tc.tile_pool(name="psum", bufs=2, space="PSUM") as ppool,
        ):
            idx_sb = cpool.tile([P, IDXCOLS], mybir.dt.int16)
            dstl_sb = cpool.tile([P, NCH], F16)
            rdeg_sb = cpool.tile([P, TILES], F32)
            xdstT_sb = cpool.tile([P, TILES * P], F16)
            iota_sb = cpool.tile([P, P], F16)
            wn_sb = cpool.tile([D, OUT], F16)
            ws_sb = cpool.tile([D, OUT], F16)
            bsum_sb = cpool.tile([1, OUT], F16)
            ones_row = cpool.tile([1, P], F16)
            ident_sb = cpool.tile([P, P], F32)
            cols_g0 = max(g[4] + g[3] * 8 for g in gathers if g[0] == 0)
            nc.sync.dma_start(out=idx_sb[:, :cols_g0], in_=idx_d[:, :cols_g0])
            nc.sync.dma_start(out=idx_sb[:, cols_g0:], in_=idx_d[:, cols_g0:])
            nc.sync.dma_start(out=dstl_sb[:], in_=dstl_d[:])
            nc.sync.dma_start(out=iota_sb[:], in_=iota_d[:])
            nc.scalar.dma_start(out=rdeg_sb[:], in_=rdeg_d[:])
            nc.scalar.dma_start(out=xdstT_sb[:], in_=xdstT_d[:])
            nc.scalar.dma_start(out=wn_sb[:], in_=wn_d[:])
            nc.scalar.dma_start(out=ws_sb[:], in_=ws_d[:])
            nc.scalar.dma_start(out=bsum_sb[:], in_=bsum_d[:])
            nc.vector.memset(ones_row[:], 1.0)
            make_identity(nc, ident_sb[:])

            def _emit_tile(t, g_sb):
                ke = KE[t]
                # batched one-hot: oh[p, k*128+j] = (iota[p,j] == dstl[p,cb+k])
                oh_sb = wpool.tile([P, KEMAX * P], DTAB, tag="oh", name=f"oh{t}")
                i_ap = iota_sb[:]
                iota3d = bass.AP(i_ap.tensor, i_ap.offset,
                                 [i_ap.ap[0], [0, ke], [i_ap.ap[1][0], P]])
                d_ap = dstl_sb[:]
                dstl3d = bass.AP(d_ap.tensor, d_ap.offset + int(cbase[t]),
                                 [d_ap.ap[0], [d_ap.ap[1][0], ke], [0, P]])
                oh3d = bass.AP(oh_sb[:].tensor, oh_sb[:].offset,
                               [oh_sb[:].ap[0], [P, ke], [1, P]])
                nc.vector.tensor_tensor(out=oh3d, in0=iota3d, in1=dstl3d,
                                        op=mybir.AluOpType.is_equal)

                ps1 = ppool.tile([P, D], F32, tag="ps1", name=f"ps1_{t}",
                                 space="PSUM", bufs=3)
                def gchunk(k):
                    if k < KL[t]:
                        return chunk_off[(t, 0)] + k
                    return chunk_off[(t, 1)] + k - KL[t]
                for k in range(ke):
                    gk = gchunk(k)
                    nc.tensor.matmul(
                        out=ps1[:, 0:D],
                        lhsT=oh_sb[:, k * P:(k + 1) * P],
                        rhs=g_sb[:, gk * P:(gk + 1) * P],
                        start=(k == 0), stop=(k == ke - 1))

                # agg = sums * (1/deg): per-partition scale on the scalar eng
                agg_sb = wpool.tile([P, D], F32, tag="agg", name=f"agg{t}")
                nc.scalar.mul(out=agg_sb[:], in_=ps1[:, 0:D],
                              mul=rdeg_sb[:, t:t + 1])
                ps_t = ppool.tile([P, P], F32, tag="pst", name=f"pst{t}",
                                  space="PSUM", bufs=3)
                nc.tensor.transpose(out=ps_t[:], in_=agg_sb[:],
                                    identity=ident_sb[:])
                aggT_sb = wpool.tile([P, D], F16, tag="aggT", name=f"agT{t}")
                nc.vector.tensor_copy(out=aggT_sb[:], in_=ps_t[:])

                ps2 = ppool.tile([P, OUT], F32, tag="ps2", name=f"ps2_{t}",
                                 space="PSUM")
                nc.tensor.matmul(out=ps2[:], lhsT=aggT_sb[:], rhs=wn_sb[:],
                                 start=True, stop=False)
                nc.tensor.matmul(out=ps2[:],
                                 lhsT=xdstT_sb[:, t * P:(t + 1) * P],
                                 rhs=ws_sb[:], start=False, stop=False)
                nc.tensor.matmul(out=ps2[:], lhsT=ones_row[:], rhs=bsum_sb[:],
                                 start=False, stop=True)
                o_sb = wpool.tile([P, OUT], F16, tag="osb", name=f"o{t}")
                nc.scalar.copy(out=o_sb[:], in_=ps2[:])
                rows = min(P, DST_PER_CORE - t * P)
                nc.sync.dma_start(out=out_d[t * P:t * P + rows, :],
                                  in_=o_sb[:rows, :])

            gq = [0]
            g_by_quad = [[] for _ in range(len(QUADS))]
            for g in gathers:
                g_by_quad[g[0]].append(g)
            for qi, qts in enumerate(QUADS):
                g_sb = wpool.tile([P, KQMAX * P], DTAB, tag="g", name=f"g{qi}", bufs=3)
                for (_, half, off, k, colb) in g_by_quad[qi]:
                    t_ap = g_sb[:]
                    out3d = bass.AP(t_ap.tensor, t_ap.offset + off * P,
                                    [t_ap.ap[0], [P, k], [1, P]])
                    nc.gpsimd.dma_gather(
                        out3d,
                        (xlo_d if half == 0 else xhi_d)[:],
                        idx_sb[:, colb:colb + k * 8],
                        k * P,
                        k * P,
                        D,
                        queue_num=(gq[0] % 4),
                    )
                    gq[0] += 1
                for t in qts:
                    _emit_tile(t, g_sb)

    nc.finalize()

    in_maps = [{
        "xlo": x_lo, "xhi": x_hi, "idx": idx_all[c], "dstl": dstl_all[c],
        "rdeg": rdeg_all[c], "xdstT": xdstT[c], "iota": iota, "wn": wn,
        "ws": ws, "bsum": bsum,
    } for c in range(N_CORES)]

    import os
    trace = os.environ.get("BSAGE_TRACE", "0") == "1"
    res = run_bass_kernel_spmd(nc, in_maps, core_ids=list(range(N_CORES)),
                               trace=trace)
    out = np.zeros((N_DST, OUT), np.float32)
    for c in range(N_CORES):
        out[core_dst_ids[c]] = res.results[c]["out"].astype(np.float32)
    if trace:
        build_and_run.last_exec_ns = res.exec_time_ns
    return out


def kernel(x_src, x_dst, edge_src, edge_dst, num_dst, W_neigh, b_neigh,
           W_self, b_self):
    x_src = np.asarray(x_src, dtype=np.float32)
    x_dst = np.asarray(x_dst, dtype=np.float32)
    edge_src = np.asarray(edge_src).astype(np.int64)
    edge_dst = np.asarray(edge_dst).astype(np.int64)
    W_neigh = np.asarray(W_neigh, dtype=np.float32)
    b_neigh = np.asarray(b_neigh, dtype=np.float32)
    W_self = np.asarray(W_self, dtype=np.float32)
    b_self = np.asarray(b_self, dtype=np.float32)
    return build_and_run(x_src, x_dst, edge_src, edge_dst, W_neigh, b_neigh,
                         W_self, b_self)


# revision 21
# speedup vs baseline: 1.1538x; 1.0898x over previous
"""BipartiteSAGEConv Trainium2 kernel.

Strategy: destination-sharded, zero collectives.
- Host: sort/partition edges by destination across 8 cores (6250 dsts each),
  group per 128-dst tile, split by src half (int16 index limit), pad to
  128-edge chunks (uniform chunk structure across cores so one SPMD program
  serves all 8 cores). Pad indices are -1 (skipped by gather ucode).
  Per-dst 1/deg is precomputed on host and uploaded (no count matmuls).
- Device per core: dma_gather (MoE row-gather ucode) pulls per-edge src rows
  HBM->SBUF in f16; scatter-add via one-hot matmul (f16) on the TensorEngine
  accumulates [dst,128] sums in PSUM; scale by 1/deg; transpose; two linear
  layers + bias via PE matmuls (all f16 operands, f32 PSUM); DMA out the
  [6250,128] f32 shard.
"""

import sys
import types

import numpy as np

N_SRC = 50000
N_DST = 50000
E = 800000
D = 128
OUT = 128
N_CORES = 8
P = 128
DST_PER_CORE = N_DST // N_CORES          # 6250
TILES = (DST_PER_CORE + P - 1) // P      # 49
HALF = 25000                             # int16 index limit split
# SWDGE ring limit: 1024 rows/gather (1920 wedges the device: NRT 101).
MAX_ROWS_PER_GATHER = 1024


def _install_ntff_hook():
    try:
        import antenv
        if "antenv.axon_hooks" in sys.modules:
            return
        mod = types.ModuleType("antenv.axon_hooks")
        _h = [None]
        mod.set_axon_ntff_profile_hook = lambda h: _h.__setitem__(0, h)
        mod.get_axon_ntff_profile_hook = lambda: _h[0]
        sys.modules["antenv.axon_hooks"] = mod
        antenv.axon_hooks = mod
        from trn_agent_boot.trn_boot import _ntff_profile_via_ctypes
        mod.set_axon_ntff_profile_hook(
            _ntff_profile_via_ctypes("/opt/axon/libaxon_pjrt.so"))
    except Exception:
        pass


def _balance_dsts(edge_src, edge_dst):
    """Assign each dst to a (core, tile, slot) bin so that per-(bin, src-half)
    edge counts are balanced (target <=1024 = 8 chunks of 128).

    Returns (dst_core, dst_tile, dst_slot, bins) where bins[c][t] is the
    ordered list of dst ids in that bin.
    """
    n_bins = N_CORES * TILES                     # 392
    lo_deg = np.bincount(edge_dst[edge_src < HALF], minlength=N_DST)
    hi_deg = np.bincount(edge_dst[edge_src >= HALF], minlength=N_DST)
    # bin capacities: last tile of each core holds the 6250-48*128=106 rest
    cap = np.full(n_bins, P, np.int64)
    cap[TILES - 1::TILES] = DST_PER_CORE - (TILES - 1) * P   # 106
    order = np.argsort(-(lo_deg + hi_deg), kind="stable")
    bin_lo = np.zeros(n_bins, np.int64)
    bin_hi = np.zeros(n_bins, np.int64)
    bin_cnt = np.zeros(n_bins, np.int64)
    members = [[] for _ in range(n_bins)]
    full = np.zeros(n_bins, bool)
    lim = 8 * P                                  # 1024-edge half target
    for d in order:
        nlo = bin_lo + lo_deg[d]
        nhi = bin_hi + hi_deg[d]
        load = np.maximum(nlo, nhi).astype(np.float64)
        load += 1e6 * (np.maximum(nlo - lim, 0) + np.maximum(nhi - lim, 0))
        load[full] = np.inf
        b = int(np.argmin(load))
        members[b].append(int(d))
        bin_lo[b] += lo_deg[d]
        bin_hi[b] += hi_deg[d]
        bin_cnt[b] += 1
        if bin_cnt[b] >= cap[b]:
            full[b] = True
    # swap-repair: force every (bin, half) load <= lim so all tiles use
    # exactly 8 chunks per half (uniform program, minimal gather rows)
    for _ in range(1000):
        over_lo = bin_lo > lim
        over_hi = bin_hi > lim
        if not (over_lo.any() or over_hi.any()):
            break
        use_lo = over_lo.any() and (not over_hi.any()
                                    or bin_lo.max() >= bin_hi.max())
        load = bin_lo if use_lo else bin_hi
        degv = lo_deg if use_lo else hi_deg
        b = int(np.argmax(load))
        b2 = int(np.argmin(load))
        mb = members[b]
        m2 = members[b2]
        d = max(mb, key=lambda x: degv[x])
        d2 = min(m2, key=lambda x: degv[x])
        if degv[d] <= degv[d2]:
            break
        mb[mb.index(d)] = d2
        m2[m2.index(d2)] = d
        bin_lo[b] += lo_deg[d2] - lo_deg[d]
        bin_hi[b] += hi_deg[d2] - hi_deg[d]
        bin_lo[b2] += lo_deg[d] - lo_deg[d2]
        bin_hi[b2] += hi_deg[d] - hi_deg[d2]

    dst_core = np.empty(N_DST, np.int64)
    dst_tile = np.empty(N_DST, np.int64)
    dst_slot = np.empty(N_DST, np.int64)
    bins = [[None] * TILES for _ in range(N_CORES)]
    for b in range(n_bins):
        c, t = divmod(b, TILES)
        ids = np.array(members[b], np.int64)
        bins[c][t] = ids
        dst_core[ids] = c
        dst_tile[ids] = t
        dst_slot[ids] = np.arange(len(ids))
    return dst_core, dst_tile, dst_slot, bins


def _prep_core(edge_src, edge_dst, core, dst_core, dst_tile, dst_slot):
    """Per-core edge structure: for each (tile, half) return the edge lists.

    Returns list over 49 tiles of (src_lo, dstl_lo, src_hi, dstl_hi) where
    src_* are int64 source indices (absolute) and dstl_* are slot-in-tile ids.
    """
    m = dst_core[edge_dst] == core
    es = edge_src[m]
    tid = dst_tile[edge_dst[m]]
    dl = dst_slot[edge_dst[m]]
    order = np.argsort(tid, kind="stable")
    es, tid, dl = es[order], tid[order], dl[order]
    bounds = np.searchsorted(tid, np.arange(TILES + 1))
    tiles = []
    for t in range(TILES):
        a, b = bounds[t], bounds[t + 1]
        s, d = es[a:b], dl[a:b]
        is_lo = s < HALF
        tiles.append((s[is_lo], d[is_lo], s[~is_lo] - HALF, d[~is_lo]))
    return tiles


def _pad_chunks(src, dstl, n_chunks):
    """Pad to n_chunks*128 edges; pad idx=0 (valid row), dstl=-1 (no one-hot)."""
    n = n_chunks * P
    s = np.zeros(n, np.int16)
    d = np.full(n, -1.0, np.float32)
    s[:len(src)] = src.astype(np.int16)
    d[:len(dstl)] = dstl.astype(np.float32)
    return s, d


def _wrap_idx(idx_flat):
    """dma_gather wrapped index layout: index j at partition j%16, col j//16,
    replicated across the 8 gpsimd cores (partition groups of 16)."""
    n = len(idx_flat)
    w = idx_flat.reshape(n // 16, 16).T          # [16, n/16]
    return np.tile(w, (8, 1))                    # [128, n/16]


def build_and_run(x_src, x_dst, edge_src, edge_dst, W_neigh, b_neigh,
                  W_self, b_self):
    _install_ntff_hook()
    from concourse import bacc, bass, mybir, tile
    from concourse.bass_utils import run_bass_kernel_spmd
    from concourse.masks import make_identity

    F32 = mybir.dt.float32
    F16 = mybir.dt.float16
    import os as _os
    use_f16 = _os.environ.get("BSAGE_F32", "0") != "1"
    DTAB = F16 if use_f16 else F32
    np_tab = np.float16 if use_f16 else np.float32

    # ---------- host-side sharding / layout ----------
    dst_core, dst_tile, dst_slot, bins = _balance_dsts(edge_src, edge_dst)
    per_core_tiles = [
        _prep_core(edge_src, edge_dst, c, dst_core, dst_tile, dst_slot)
        for c in range(N_CORES)]
    # dst ids of core c in output-row order
    core_dst_ids = [np.concatenate(bins[c]) for c in range(N_CORES)]

    # per-dst reciprocal degree, tile-major per core: rdeg[c][p, t]
    deg = np.bincount(edge_dst, minlength=N_DST).astype(np.float32)
    rdeg_full = 1.0 / np.maximum(deg, 1.0)
    rdeg_all = np.zeros((N_CORES, P, TILES), np.float32)
    for c in range(N_CORES):
        shard = np.zeros(TILES * P, np.float32)
        for t in range(TILES):
            ids = bins[c][t]
            shard[t * P:t * P + len(ids)] = rdeg_full[ids]
        rdeg_all[c] = shard.reshape(TILES, P).T

    # uniform chunk counts across cores (SPMD: one program, 8 data sets)
    KL = [max(max(1, -(-len(per_core_tiles[c][t][0]) // P))
              for c in range(N_CORES)) for t in range(TILES)]
    KH = [max(max(1, -(-len(per_core_tiles[c][t][2]) // P))
              for c in range(N_CORES)) for t in range(TILES)]
    KE = [KL[t] + KH[t] for t in range(TILES)]
    NCH = sum(KE)                                 # total chunks per core
    KEMAX = max(KE)

    # quad grouping: a few tiles share one g buffer; chunk layout within a
    # quad: [lo(t0)|lo(t1)|...|hi(t0)|hi(t1)|...]
    _sizes = [8] * ((TILES - 9) // 8) + [4, 2, 2, 1]
    _rem = TILES - sum(_sizes)
    _sizes = [8] * (_rem // 8) + ([_rem % 8] if _rem % 8 else []) + _sizes if _rem > 0 else _sizes
    QUADS = []
    _q = 0
    for _s in _sizes:
        QUADS.append(list(range(_q, _q + _s)))
        _q += _s
    assert _q == TILES, (_q, TILES, _sizes)
    # chunk offset of each (tile, half) within its quad buffer
    chunk_off = {}
    quad_chunks = []
    for qi, qts in enumerate(QUADS):
        off = 0
        for t in qts:
            chunk_off[(t, 0)] = off
            off += KL[t]
        for t in qts:
            chunk_off[(t, 1)] = off
            off += KH[t]
        quad_chunks.append(off)
    KQMAX = max(quad_chunks)

    # gather plan: per quad per half, one contiguous chunk span covering the
    # member tiles' chunks, split into <=8-chunk (1024-row) instructions.
    # gathers: (quad, half, chunk_off_in_quad, n_chunks, idx_col_base)
    gathers = []
    idx_cols = 0                                  # int16 columns consumed
    for qi, qts in enumerate(QUADS):
        for half in (0, 1):
            span = sum((KL if half == 0 else KH)[t] for t in qts)
            base = chunk_off[(qts[0], half)]
            k_done = 0
            while k_done < span:
                k = min(span - k_done, MAX_ROWS_PER_GATHER // P)
                gathers.append((qi, half, base + k_done, k, idx_cols))
                idx_cols += k * 8
                k_done += k
    IDXCOLS = idx_cols

    # per-core data arrays
    idx_all = np.zeros((N_CORES, P, IDXCOLS), np.int16)
    dstl_all = np.zeros((N_CORES, P, NCH), np.float16)
    cbase = np.concatenate([[0], np.cumsum(KE)])  # chunk col base per tile
    for c in range(N_CORES):
        for t in range(TILES):
            s_lo, d_lo, s_hi, d_hi = per_core_tiles[c][t]
            sl, dl = _pad_chunks(s_lo, d_lo, KL[t])
            sh, dh = _pad_chunks(s_hi, d_hi, KH[t])
            d_cat = np.concatenate([dl, dh])
            # dstl layout: [128, NCH]; slot p of chunk k = edge k*128+p
            dstl_all[c][:, cbase[t]:cbase[t + 1]] = (
                d_cat.reshape(KE[t], P).T.astype(np.float16))
        # per-quad padded source-index streams (chunk layout order)
        quad_src = []
        for qi, qts in enumerate(QUADS):
            parts = []
            for t in qts:
                s_lo, d_lo, _, _ = per_core_tiles[c][t]
                parts.append(_pad_chunks(s_lo, d_lo, KL[t])[0])
            for t in qts:
                _, _, s_hi, d_hi = per_core_tiles[c][t]
                parts.append(_pad_chunks(s_hi, d_hi, KH[t])[0])
            quad_src.append(np.concatenate(parts))
        for (qi, half, off, k, colb) in gathers:
            rows = quad_src[qi][off * P:(off + k) * P]
            idx_all[c][:, colb:colb + k * 8] = _wrap_idx(rows)

    x_lo = np.ascontiguousarray(x_src[:HALF]).astype(np_tab)
    x_hi = np.ascontiguousarray(x_src[HALF:]).astype(np_tab)
    xdstT = np.zeros((N_CORES, P, TILES * P), np.float16)
    for c in range(N_CORES):
        for t in range(TILES):
            ids = bins[c][t]
            xdstT[c][:, t * P:t * P + len(ids)] = (
                x_dst[ids].T.astype(np.float16))
    iota = np.tile(np.arange(P, dtype=np.float16), (P, 1))
    wn = W_neigh.astype(np.float16)
    ws = W_self.astype(np.float16)
    bsum = (b_neigh + b_self).astype(np.float16)[None, :]  # [1,128]

    # ---------- device program ----------
    nc = bacc.Bacc("TRN2", target_bir_lowering=False, debug=False,
                   num_devices=N_CORES, num_swdge_queues=4)
    xlo_d = nc.dram_tensor("xlo", [HALF, D], DTAB, kind="ExternalInput").ap()
    xhi_d = nc.dram_tensor("xhi", [HALF, D], DTAB, kind="ExternalInput").ap()
    idx_d = nc.dram_tensor("idx", [P, IDXCOLS], mybir.dt.int16,
                           kind="ExternalInput").ap()
    dstl_d = nc.dram_tensor("dstl", [P, NCH], F16, kind="ExternalInput").ap()
    rdeg_d = nc.dram_tensor("rdeg", [P, TILES], F32, kind="ExternalInput").ap()
    xdstT_d = nc.dram_tensor("xdstT", [P, TILES * P], F16,
                             kind="ExternalInput").ap()
    iota_d = nc.dram_tensor("iota", [P, P], F16, kind="ExternalInput").ap()
    wn_d = nc.dram_tensor("wn", [D, OUT], F16, kind="ExternalInput").ap()
    ws_d = nc.dram_tensor("ws", [D, OUT], F16, kind="ExternalInput").ap()
    bsum_d = nc.dram_tensor("bsum", [1, OUT], F16, kind="ExternalInput").ap()
    out_d = nc.dram_tensor("out", [DST_PER_CORE, OUT], F16,
                           kind="ExternalOutput").ap()

    with tile.TileContext(nc) as tc:
        with (
            tc.tile_pool(name="const", bufs=1) as cpool,
            tc.tile_pool(name="work", bufs=4) as wpool,
            tc.tile_pool(name="psum", bufs=2, space="PSUM") as ppool,
        ):
            idx_sb = cpool.tile([P, IDXCOLS], mybir.dt.int16)
            dstl_sb = cpool.tile([P, NCH], F16)
            rdeg_sb = cpool.tile([P, TILES], F32)
            xdstT_sb = cpool.tile([P, TILES * P], F16)
            iota_sb = cpool.tile([P, P], F16)
            wn_sb = cpool.tile([D, OUT], F16)
            ws_sb = cpool.tile([D, OUT], F16)
            bsum_sb = cpool.tile([1, OUT], F16)
            ones_row = cpool.tile([1, P], F16)
            ident_sb = cpool.tile([P, P], F32)
            cols_g0 = max(g[4] + g[3] * 8 for g in gathers if g[0] == 0)
            nc.sync.dma_start(out=idx_sb[:, :cols_g0], in_=idx_d[:, :cols_g0])
            nc.sync.dma_start(out=idx_sb[:, cols_g0:], in_=idx_d[:, cols_g0:])
            nc.sync.dma_start(out=dstl_sb[:], in_=dstl_d[:])
            nc.sync.dma_start(out=iota_sb[:], in_=iota_d[:])
            nc.scalar.dma_start(out=rdeg_sb[:], in_=rdeg_d[:])
            nc.scalar.dma_start(out=xdstT_sb[:], in_=xdstT_d[:])
            nc.scalar.dma_start(out=wn_sb[:], in_=wn_d[:])
            nc.scalar.dma_start(out=ws_sb[:], in_=ws_d[:])
            nc.scalar.dma_start(out=bsum_sb[:], in_=bsum_d[:])
            nc.vector.memset(ones_row[:], 1.0)
            make_identity(nc, ident_sb[:])

            def _emit_tile(t, g_sb):
                ke = KE[t]
                # batched one-hot: oh[p, k*128+j] = (iota[p,j] == dstl[p,cb+k])
                oh_sb = wpool.tile([P, KEMAX * P], DTAB, tag="oh", name=f"oh{t}")
                i_ap = iota_sb[:]
                iota3d = bass.AP(i_ap.tensor, i_ap.offset,
                                 [i_ap.ap[0], [0, ke], [i_ap.ap[1][0], P]])
                d_ap = dstl_sb[:]
                dstl3d = bass.AP(d_ap.tensor, d_ap.offset + int(cbase[t]),
                                 [d_ap.ap[0], [d_ap.ap[1][0], ke], [0, P]])
                oh3d = bass.AP(oh_sb[:].tensor, oh_sb[:].offset,
                               [oh_sb[:].ap[0], [P, ke], [1, P]])
                nc.vector.tensor_tensor(out=oh3d, in0=iota3d, in1=dstl3d,
                                        op=mybir.AluOpType.is_equal)

                ps1 = ppool.tile([P, D], F32, tag="ps1", name=f"ps1_{t}",
                                 space="PSUM", bufs=3)
                def gchunk(k):
                    if k < KL[t]:
                        return chunk_off[(t, 0)] + k
                    return chunk_off[(t, 1)] + k - KL[t]
                for k in range(ke):
                    gk = gchunk(k)
                    nc.tensor.matmul(
                        out=ps1[:, 0:D],
                        lhsT=oh_sb[:, k * P:(k + 1) * P],
                        rhs=g_sb[:, gk * P:(gk + 1) * P],
                        start=(k == 0), stop=(k == ke - 1))

                # agg = sums * (1/deg): per-partition scale on the scalar eng
                agg_sb = wpool.tile([P, D], F32, tag="agg", name=f"agg{t}")
                nc.scalar.mul(out=agg_sb[:], in_=ps1[:, 0:D],
                              mul=rdeg_sb[:, t:t + 1])
                ps_t = ppool.tile([P, P], F32, tag="pst", name=f"pst{t}",
                                  space="PSUM", bufs=3)
                nc.tensor.transpose(out=ps_t[:], in_=agg_sb[:],
                                    identity=ident_sb[:])
                aggT_sb = wpool.tile([P, D], F16, tag="aggT", name=f"agT{t}")
                nc.vector.tensor_copy(out=aggT_sb[:], in_=ps_t[:])

                ps2 = ppool.tile([P, OUT], F32, tag="ps2", name=f"ps2_{t}",
                                 space="PSUM")
                nc.tensor.matmul(out=ps2[:], lhsT=aggT_sb[:], rhs=wn_sb[:],
                                 start=True, stop=False)
                nc.tensor.matmul(out=ps2[:],
                                 lhsT=xdstT_sb[:, t * P:(t + 1) * P],
                                 rhs=ws_sb[:], start=False, stop=False)
                nc.tensor.matmul(out=ps2[:], lhsT=ones_row[:], rhs=bsum_sb[:],
                                 start=False, stop=True)
                o_sb = wpool.tile([P, OUT], F16, tag="osb", name=f"o{t}")
                nc.scalar.copy(out=o_sb[:], in_=ps2[:])
                rows = min(P, DST_PER_CORE - t * P)
                nc.sync.dma_start(out=out_d[t * P:t * P + rows, :],
                                  in_=o_sb[:rows, :])

            gq = [0]
            g_by_quad = [[] for _ in range(len(QUADS))]
            for g in gathers:
                g_by_quad[g[0]].append(g)
            for qi, qts in enumerate(QUADS):
                g_sb = wpool.tile([P, KQMAX * P], DTAB, tag="g", name=f"g{qi}", bufs=3)
                for (_, half, off, k, colb) in g_by_quad[qi]:
                    t_ap = g_sb[:]
                    out3d = bass.AP(t_ap.tensor, t_ap.offset + off * P,
                                    [t_ap.ap[0], [P, k], [1, P]])
                    nc.gpsimd.dma_gather(
                        out3d,
                        (xlo_d if half == 0 else xhi_d)[:],
                        idx_sb[:, colb:colb + k * 8],
                        k * P,
                        k * P,
                        D,
                        queue_num=(gq[0] % 4),
                    )
                    gq[0] += 1
                for t in qts:
                    _emit_tile(t, g_sb)

    nc.finalize()

    in_maps = [{
        "xlo": x_lo, "xhi": x_hi, "idx": idx_all[c], "dstl": dstl_all[c],
        "rdeg": rdeg_all[c], "xdstT": xdstT[c], "iota": iota, "wn": wn,
        "ws": ws, "bsum": bsum,
    } for c in range(N_CORES)]

    import os
    trace = os.environ.get("BSAGE_TRACE", "0") == "1"
    res = run_bass_kernel_spmd(nc, in_maps, core_ids=list(range(N_CORES)),
                               trace=trace)
    out = np.zeros((N_DST, OUT), np.float32)
    for c in range(N_CORES):
        out[core_dst_ids[c]] = res.results[c]["out"].astype(np.float32)
    if trace:
        build_and_run.last_exec_ns = res.exec_time_ns
    return out


def kernel(x_src, x_dst, edge_src, edge_dst, num_dst, W_neigh, b_neigh,
           W_self, b_self):
    x_src = np.asarray(x_src, dtype=np.float32)
    x_dst = np.asarray(x_dst, dtype=np.float32)
    edge_src = np.asarray(edge_src).astype(np.int64)
    edge_dst = np.asarray(edge_dst).astype(np.int64)
    W_neigh = np.asarray(W_neigh, dtype=np.float32)
    b_neigh = np.asarray(b_neigh, dtype=np.float32)
    W_self = np.asarray(W_self, dtype=np.float32)
    b_self = np.asarray(b_self, dtype=np.float32)
    return build_and_run(x_src, x_dst, edge_src, edge_dst, W_neigh, b_neigh,
                         W_self, b_self)


# revision 27
# speedup vs baseline: 1.1593x; 1.0048x over previous
"""BipartiteSAGEConv Trainium2 kernel.

Strategy: destination-sharded, zero collectives.
- Host: degree-balanced bin-packing assigns each dst to a (core, tile, slot)
  bin so that every (core, tile, src-half) holds <=1024 edges -> a perfectly
  uniform 8-chunk-per-half layout (minimal gather rows, one SPMD program for
  all 8 cores). Src split in two halves (int16 gather-index limit). Per-dst
  1/deg is precomputed on host and uploaded (no count matmuls).
- Device per core: dma_gather (MoE row-gather ucode) pulls per-edge src rows
  HBM->SBUF in f16; scatter-add via one-hot matmul (f16) on the TensorEngine
  accumulates [dst,128] sums in PSUM; scale by 1/deg; transpose; two linear
  layers + bias via PE matmuls (all f16 operands, f32 PSUM); DMA out the
  [6250,128] f32 shard.
"""

import sys
import types

import numpy as np

N_SRC = 50000
N_DST = 50000
E = 800000
D = 128
OUT = 128
N_CORES = 8
P = 128
DST_PER_CORE = N_DST // N_CORES          # 6250
TILES = (DST_PER_CORE + P - 1) // P      # 49
HALF = 25000                             # int16 index limit split
# SWDGE ring limit: 1024 rows/gather (1920 wedges the device: NRT 101).
MAX_ROWS_PER_GATHER = 1024


def _install_ntff_hook():
    try:
        import antenv
        if "antenv.axon_hooks" in sys.modules:
            return
        mod = types.ModuleType("antenv.axon_hooks")
        _h = [None]
        mod.set_axon_ntff_profile_hook = lambda h: _h.__setitem__(0, h)
        mod.get_axon_ntff_profile_hook = lambda: _h[0]
        sys.modules["antenv.axon_hooks"] = mod
        antenv.axon_hooks = mod
        from trn_agent_boot.trn_boot import _ntff_profile_via_ctypes
        mod.set_axon_ntff_profile_hook(
            _ntff_profile_via_ctypes("/opt/axon/libaxon_pjrt.so"))
    except Exception:
        pass


def _balance_dsts(edge_src, edge_dst):
    """Assign each dst to a (core, tile, slot) bin so that per-(bin, src-half)
    edge counts are balanced (target <=1024 = 8 chunks of 128).

    Returns (dst_core, dst_tile, dst_slot, bins) where bins[c][t] is the
    ordered list of dst ids in that bin.
    """
    n_bins = N_CORES * TILES                     # 392
    lo_deg = np.bincount(edge_dst[edge_src < HALF], minlength=N_DST)
    hi_deg = np.bincount(edge_dst[edge_src >= HALF], minlength=N_DST)
    # bin capacities: last tile of each core holds the 6250-48*128=106 rest
    cap = np.full(n_bins, P, np.int64)
    cap[TILES - 1::TILES] = DST_PER_CORE - (TILES - 1) * P   # 106
    order = np.argsort(-(lo_deg + hi_deg), kind="stable")
    bin_lo = np.zeros(n_bins, np.int64)
    bin_hi = np.zeros(n_bins, np.int64)
    bin_cnt = np.zeros(n_bins, np.int64)
    members = [[] for _ in range(n_bins)]
    full = np.zeros(n_bins, bool)
    lim = 8 * P                                  # 1024-edge half target
    for d in order:
        nlo = bin_lo + lo_deg[d]
        nhi = bin_hi + hi_deg[d]
        load = np.maximum(nlo, nhi).astype(np.float64)
        load += 1e6 * (np.maximum(nlo - lim, 0) + np.maximum(nhi - lim, 0))
        load[full] = np.inf
        b = int(np.argmin(load))
        members[b].append(int(d))
        bin_lo[b] += lo_deg[d]
        bin_hi[b] += hi_deg[d]
        bin_cnt[b] += 1
        if bin_cnt[b] >= cap[b]:
            full[b] = True
    # swap-repair: force every (bin, half) load <= lim so all tiles use
    # exactly 8 chunks per half (uniform program, minimal gather rows)
    for _ in range(1000):
        over_lo = bin_lo > lim
        over_hi = bin_hi > lim
        if not (over_lo.any() or over_hi.any()):
            break
        use_lo = over_lo.any() and (not over_hi.any()
                                    or bin_lo.max() >= bin_hi.max())
        load = bin_lo if use_lo else bin_hi
        degv = lo_deg if use_lo else hi_deg
        b = int(np.argmax(load))
        b2 = int(np.argmin(load))
        mb = members[b]
        m2 = members[b2]
        d = max(mb, key=lambda x: degv[x])
        d2 = min(m2, key=lambda x: degv[x])
        if degv[d] <= degv[d2]:
            break
        mb[mb.index(d)] = d2
        m2[m2.index(d2)] = d
        bin_lo[b] += lo_deg[d2] - lo_deg[d]
        bin_hi[b] += hi_deg[d2] - hi_deg[d]
        bin_lo[b2] += lo_deg[d] - lo_deg[d2]
        bin_hi[b2] += hi_deg[d] - hi_deg[d2]

    dst_core = np.empty(N_DST, np.int64)
    dst_tile = np.empty(N_DST, np.int64)
    dst_slot = np.empty(N_DST, np.int64)
    bins = [[None] * TILES for _ in range(N_CORES)]
    for b in range(n_bins):
        c, t = divmod(b, TILES)
        ids = np.array(members[b], np.int64)
        bins[c][t] = ids
        dst_core[ids] = c
        dst_tile[ids] = t
        dst_slot[ids] = np.arange(len(ids))
    return dst_core, dst_tile, dst_slot, bins


def _prep_core(edge_src, edge_dst, core, dst_core, dst_tile, dst_slot):
    """Per-core edge structure: for each (tile, half) return the edge lists.

    Returns list over 49 tiles of (src_lo, dstl_lo, src_hi, dstl_hi) where
    src_* are int64 source indices (absolute) and dstl_* are slot-in-tile ids.
    """
    m = dst_core[edge_dst] == core
    es = edge_src[m]
    tid = dst_tile[edge_dst[m]]
    dl = dst_slot[edge_dst[m]]
    order = np.argsort(tid, kind="stable")
    es, tid, dl = es[order], tid[order], dl[order]
    bounds = np.searchsorted(tid, np.arange(TILES + 1))
    tiles = []
    for t in range(TILES):
        a, b = bounds[t], bounds[t + 1]
        s, d = es[a:b], dl[a:b]
        is_lo = s < HALF
        tiles.append((s[is_lo], d[is_lo], s[~is_lo] - HALF, d[~is_lo]))
    return tiles


def _pad_chunks(src, dstl, n_chunks):
    """Pad to n_chunks*128 edges; pad idx=0 (valid row), dstl=-1 (no one-hot)."""
    n = n_chunks * P
    s = np.zeros(n, np.int16)
    d = np.full(n, -1.0, np.float32)
    s[:len(src)] = src.astype(np.int16)
    d[:len(dstl)] = dstl.astype(np.float32)
    return s, d


def _wrap_idx(idx_flat):
    """dma_gather wrapped index layout: index j at partition j%16, col j//16,
    replicated across the 8 gpsimd cores (partition groups of 16)."""
    n = len(idx_flat)
    w = idx_flat.reshape(n // 16, 16).T          # [16, n/16]
    return np.tile(w, (8, 1))                    # [128, n/16]


def build_and_run(x_src, x_dst, edge_src, edge_dst, W_neigh, b_neigh,
                  W_self, b_self):
    _install_ntff_hook()
    from concourse import bacc, bass, mybir, tile
    from concourse.bass_utils import run_bass_kernel_spmd
    from concourse.masks import make_identity

    F32 = mybir.dt.float32
    F16 = mybir.dt.float16
    import os as _os
    use_f16 = _os.environ.get("BSAGE_F32", "0") != "1"
    DTAB = F16 if use_f16 else F32
    np_tab = np.float16 if use_f16 else np.float32

    # ---------- host-side sharding / layout ----------
    dst_core, dst_tile, dst_slot, bins = _balance_dsts(edge_src, edge_dst)
    per_core_tiles = [
        _prep_core(edge_src, edge_dst, c, dst_core, dst_tile, dst_slot)
        for c in range(N_CORES)]
    # dst ids of core c in output-row order
    core_dst_ids = [np.concatenate(bins[c]) for c in range(N_CORES)]

    # per-dst reciprocal degree, tile-major per core: rdeg[c][p, t]
    deg = np.bincount(edge_dst, minlength=N_DST).astype(np.float32)
    rdeg_full = 1.0 / np.maximum(deg, 1.0)
    rdeg_all = np.zeros((N_CORES, P, TILES), np.float32)
    for c in range(N_CORES):
        shard = np.zeros(TILES * P, np.float32)
        for t in range(TILES):
            ids = bins[c][t]
            shard[t * P:t * P + len(ids)] = rdeg_full[ids]
        rdeg_all[c] = shard.reshape(TILES, P).T

    # uniform chunk counts across cores (SPMD: one program, 8 data sets)
    KL = [max(max(1, -(-len(per_core_tiles[c][t][0]) // P))
              for c in range(N_CORES)) for t in range(TILES)]
    KH = [max(max(1, -(-len(per_core_tiles[c][t][2]) // P))
              for c in range(N_CORES)) for t in range(TILES)]
    KE = [KL[t] + KH[t] for t in range(TILES)]
    NCH = sum(KE)                                 # total chunks per core
    KEMAX = max(KE)

    # quad grouping: a few tiles share one g buffer; chunk layout within a
    # quad: [lo(t0)|lo(t1)|...|hi(t0)|hi(t1)|...]
    _sizes = [8] * ((TILES - 9) // 8) + [4, 2, 1, 1, 1]
    _rem = TILES - sum(_sizes)
    _sizes = [8] * (_rem // 8) + ([_rem % 8] if _rem % 8 else []) + _sizes if _rem > 0 else _sizes
    QUADS = []
    _q = 0
    for _s in _sizes:
        QUADS.append(list(range(_q, _q + _s)))
        _q += _s
    assert _q == TILES, (_q, TILES, _sizes)
    # chunk offset of each (tile, half) within its quad buffer
    chunk_off = {}
    quad_chunks = []
    for qi, qts in enumerate(QUADS):
        off = 0
        for t in qts:
            chunk_off[(t, 0)] = off
            off += KL[t]
        for t in qts:
            chunk_off[(t, 1)] = off
            off += KH[t]
        quad_chunks.append(off)
    KQMAX = max(quad_chunks)

    # gather plan: per quad per half, one contiguous chunk span covering the
    # member tiles' chunks, split into <=8-chunk (1024-row) instructions.
    # gathers: (quad, half, chunk_off_in_quad, n_chunks, idx_col_base)
    gathers = []
    idx_cols = 0                                  # int16 columns consumed
    for qi, qts in enumerate(QUADS):
        for half in (0, 1):
            span = sum((KL if half == 0 else KH)[t] for t in qts)
            base = chunk_off[(qts[0], half)]
            k_done = 0
            while k_done < span:
                k = min(span - k_done, MAX_ROWS_PER_GATHER // P)
                gathers.append((qi, half, base + k_done, k, idx_cols))
                idx_cols += k * 8
                k_done += k
    IDXCOLS = idx_cols

    # per-core data arrays
    idx_all = np.zeros((N_CORES, P, IDXCOLS), np.int16)
    dstl_all = np.zeros((N_CORES, P, NCH), np.float16)
    cbase = np.concatenate([[0], np.cumsum(KE)])  # chunk col base per tile
    for c in range(N_CORES):
        for t in range(TILES):
            s_lo, d_lo, s_hi, d_hi = per_core_tiles[c][t]
            sl, dl = _pad_chunks(s_lo, d_lo, KL[t])
            sh, dh = _pad_chunks(s_hi, d_hi, KH[t])
            d_cat = np.concatenate([dl, dh])
            # dstl layout: [128, NCH]; slot p of chunk k = edge k*128+p
            dstl_all[c][:, cbase[t]:cbase[t + 1]] = (
                d_cat.reshape(KE[t], P).T.astype(np.float16))
        # per-quad padded source-index streams (chunk layout order)
        quad_src = []
        for qi, qts in enumerate(QUADS):
            parts = []
            for t in qts:
                s_lo, d_lo, _, _ = per_core_tiles[c][t]
                parts.append(_pad_chunks(s_lo, d_lo, KL[t])[0])
            for t in qts:
                _, _, s_hi, d_hi = per_core_tiles[c][t]
                parts.append(_pad_chunks(s_hi, d_hi, KH[t])[0])
            quad_src.append(np.concatenate(parts))
        for (qi, half, off, k, colb) in gathers:
            rows = quad_src[qi][off * P:(off + k) * P]
            idx_all[c][:, colb:colb + k * 8] = _wrap_idx(rows)

    x_lo = np.ascontiguousarray(x_src[:HALF]).astype(np_tab)
    x_hi = np.ascontiguousarray(x_src[HALF:]).astype(np_tab)
    xdstT = np.zeros((N_CORES, P, TILES * P), np.float16)
    for c in range(N_CORES):
        for t in range(TILES):
            ids = bins[c][t]
            xdstT[c][:, t * P:t * P + len(ids)] = (
                x_dst[ids].T.astype(np.float16))
    iota = np.tile(np.arange(P, dtype=np.float16), (P, 1))
    wn = W_neigh.astype(np.float16)
    ws = W_self.astype(np.float16)
    bsum = (b_neigh + b_self).astype(np.float16)[None, :]  # [1,128]

    # ---------- device program ----------
    nc = bacc.Bacc("TRN2", target_bir_lowering=False, debug=False,
                   num_devices=N_CORES, num_swdge_queues=4)
    xlo_d = nc.dram_tensor("xlo", [HALF, D], DTAB, kind="ExternalInput").ap()
    xhi_d = nc.dram_tensor("xhi", [HALF, D], DTAB, kind="ExternalInput").ap()
    idx_d = nc.dram_tensor("idx", [P, IDXCOLS], mybir.dt.int16,
                           kind="ExternalInput").ap()
    dstl_d = nc.dram_tensor("dstl", [P, NCH], F16, kind="ExternalInput").ap()
    rdeg_d = nc.dram_tensor("rdeg", [P, TILES], F32, kind="ExternalInput").ap()
    xdstT_d = nc.dram_tensor("xdstT", [P, TILES * P], F16,
                             kind="ExternalInput").ap()
    iota_d = nc.dram_tensor("iota", [P, P], F16, kind="ExternalInput").ap()
    wn_d = nc.dram_tensor("wn", [D, OUT], F16, kind="ExternalInput").ap()
    ws_d = nc.dram_tensor("ws", [D, OUT], F16, kind="ExternalInput").ap()
    bsum_d = nc.dram_tensor("bsum", [1, OUT], F16, kind="ExternalInput").ap()
    out_d = nc.dram_tensor("out", [DST_PER_CORE, OUT], F16,
                           kind="ExternalOutput").ap()

    with tile.TileContext(nc) as tc:
        with (
            tc.tile_pool(name="const", bufs=1) as cpool,
            tc.tile_pool(name="work", bufs=4) as wpool,
            tc.tile_pool(name="psum", bufs=2, space="PSUM") as ppool,
        ):
            # per-quad idx tiles: the first gather only depends on quad 0's
            # small idx slice, not the whole upload
            qbase = []
            qcols = []
            for qi in range(len(QUADS)):
                cols = [g[4] for g in gathers if g[0] == qi]
                ncols = [g[4] + g[3] * 8 for g in gathers if g[0] == qi]
                qbase.append(min(cols))
                qcols.append(max(ncols) - min(cols))
            idx_sbs = [cpool.tile([P, qcols[qi]], mybir.dt.int16,
                                  name=f"idxq{qi}")
                       for qi in range(len(QUADS))]
            dstl_sb = cpool.tile([P, NCH], F16)
            rdeg_sb = cpool.tile([P, TILES], F32)
            xdstT_sb = cpool.tile([P, TILES * P], F16)
            iota_sb = cpool.tile([P, P], F16)
            wn_sb = cpool.tile([D, OUT], F16)
            ws_sb = cpool.tile([D, OUT], F16)
            bsum_sb = cpool.tile([1, OUT], F16)
            ones_row = cpool.tile([1, P], F16)
            ident_sb = cpool.tile([P, P], F32)
            for qi in range(len(QUADS)):
                nc.sync.dma_start(
                    out=idx_sbs[qi][:],
                    in_=idx_d[:, qbase[qi]:qbase[qi] + qcols[qi]])
            nc.sync.dma_start(out=dstl_sb[:], in_=dstl_d[:])
            nc.sync.dma_start(out=iota_sb[:], in_=iota_d[:])
            nc.scalar.dma_start(out=rdeg_sb[:], in_=rdeg_d[:])
            nc.scalar.dma_start(out=xdstT_sb[:], in_=xdstT_d[:])
            nc.scalar.dma_start(out=wn_sb[:], in_=wn_d[:])
            nc.scalar.dma_start(out=ws_sb[:], in_=ws_d[:])
            nc.scalar.dma_start(out=bsum_sb[:], in_=bsum_d[:])
            nc.vector.memset(ones_row[:], 1.0)
            make_identity(nc, ident_sb[:])

            def _emit_tile(t, g_sb):
                ke, kl = KE[t], KL[t]
                # batched one-hot, built per src-half so the lo-half matmuls
                # only depend on the lo gather (better pipeline overlap):
                # oh[p, k*128+j] = (iota[p,j] == dstl[p,cb+k])
                oh_sb = wpool.tile([P, KEMAX * P], DTAB, tag="oh", name=f"oh{t}")
                i_ap = iota_sb[:]
                d_ap = dstl_sb[:]
                o_ap = oh_sb[:]
                ps1 = ppool.tile([P, D], F32, tag="ps1", name=f"ps1_{t}",
                                 space="PSUM", bufs=3)
                for half, k0, k1 in ((0, 0, kl), (1, kl, ke)):
                    nk = k1 - k0
                    iota3d = bass.AP(i_ap.tensor, i_ap.offset,
                                     [i_ap.ap[0], [0, nk], [i_ap.ap[1][0], P]])
                    dstl3d = bass.AP(
                        d_ap.tensor, d_ap.offset + int(cbase[t]) + k0,
                        [d_ap.ap[0], [d_ap.ap[1][0], nk], [0, P]])
                    oh3d = bass.AP(o_ap.tensor, o_ap.offset + k0 * P,
                                   [o_ap.ap[0], [P, nk], [1, P]])
                    nc.vector.tensor_tensor(out=oh3d, in0=iota3d, in1=dstl3d,
                                            op=mybir.AluOpType.is_equal)
                    gb = chunk_off[(t, half)] - k0
                    for k in range(k0, k1):
                        gk = gb + k
                        nc.tensor.matmul(
                            out=ps1[:, 0:D],
                            lhsT=oh_sb[:, k * P:(k + 1) * P],
                            rhs=g_sb[:, gk * P:(gk + 1) * P],
                            start=(k == 0), stop=(k == ke - 1))

                # agg = sums * (1/deg): per-partition scale on the scalar eng
                agg_sb = wpool.tile([P, D], F32, tag="agg", name=f"agg{t}")
                nc.scalar.mul(out=agg_sb[:], in_=ps1[:, 0:D],
                              mul=rdeg_sb[:, t:t + 1])
                ps_t = ppool.tile([P, P], F32, tag="pst", name=f"pst{t}",
                                  space="PSUM", bufs=3)
                nc.tensor.transpose(out=ps_t[:], in_=agg_sb[:],
                                    identity=ident_sb[:])
                aggT_sb = wpool.tile([P, D], F16, tag="aggT", name=f"agT{t}")
                nc.vector.tensor_copy(out=aggT_sb[:], in_=ps_t[:])

                ps2 = ppool.tile([P, OUT], F32, tag="ps2", name=f"ps2_{t}",
                                 space="PSUM")
                nc.tensor.matmul(out=ps2[:], lhsT=aggT_sb[:], rhs=wn_sb[:],
                                 start=True, stop=False)
                nc.tensor.matmul(out=ps2[:],
                                 lhsT=xdstT_sb[:, t * P:(t + 1) * P],
                                 rhs=ws_sb[:], start=False, stop=False)
                nc.tensor.matmul(out=ps2[:], lhsT=ones_row[:], rhs=bsum_sb[:],
                                 start=False, stop=True)
                o_sb = wpool.tile([P, OUT], F16, tag="osb", name=f"o{t}")
                nc.scalar.copy(out=o_sb[:], in_=ps2[:])
                rows = min(P, DST_PER_CORE - t * P)
                nc.sync.dma_start(out=out_d[t * P:t * P + rows, :],
                                  in_=o_sb[:rows, :])

            gq = [0]
            g_by_quad = [[] for _ in range(len(QUADS))]
            for g in gathers:
                g_by_quad[g[0]].append(g)
            for qi, qts in enumerate(QUADS):
                g_sb = wpool.tile([P, KQMAX * P], DTAB, tag="g", name=f"g{qi}", bufs=3)
                for (_, half, off, k, colb) in g_by_quad[qi]:
                    t_ap = g_sb[:]
                    out3d = bass.AP(t_ap.tensor, t_ap.offset + off * P,
                                    [t_ap.ap[0], [P, k], [1, P]])
                    lcol = colb - qbase[qi]
                    nc.gpsimd.dma_gather(
                        out3d,
                        (xlo_d if half == 0 else xhi_d)[:],
                        idx_sbs[qi][:, lcol:lcol + k * 8],
                        k * P,
                        k * P,
                        D,
                        queue_num=(gq[0] % 4),
                    )
                    gq[0] += 1
                for t in qts:
                    _emit_tile(t, g_sb)

    nc.finalize()

    in_maps = [{
        "xlo": x_lo, "xhi": x_hi, "idx": idx_all[c], "dstl": dstl_all[c],
        "rdeg": rdeg_all[c], "xdstT": xdstT[c], "iota": iota, "wn": wn,
        "ws": ws, "bsum": bsum,
    } for c in range(N_CORES)]

    import os
    trace = os.environ.get("BSAGE_TRACE", "0") == "1"
    res = run_bass_kernel_spmd(nc, in_maps, core_ids=list(range(N_CORES)),
                               trace=trace)
    out = np.zeros((N_DST, OUT), np.float32)
    for c in range(N_CORES):
        out[core_dst_ids[c]] = res.results[c]["out"].astype(np.float32)
    if trace:
        build_and_run.last_exec_ns = res.exec_time_ns
    return out


def kernel(x_src, x_dst, edge_src, edge_dst, num_dst, W_neigh, b_neigh,
           W_self, b_self):
    x_src = np.asarray(x_src, dtype=np.float32)
    x_dst = np.asarray(x_dst, dtype=np.float32)
    edge_src = np.asarray(edge_src).astype(np.int64)
    edge_dst = np.asarray(edge_dst).astype(np.int64)
    W_neigh = np.asarray(W_neigh, dtype=np.float32)
    b_neigh = np.asarray(b_neigh, dtype=np.float32)
    W_self = np.asarray(W_self, dtype=np.float32)
    b_self = np.asarray(b_self, dtype=np.float32)
    return build_and_run(x_src, x_dst, edge_src, edge_dst, W_neigh, b_neigh,
                         W_self, b_self)


# revision 32
# speedup vs baseline: 1.1864x; 1.0233x over previous
"""BipartiteSAGEConv Trainium2 kernel.

Strategy: destination-sharded, zero collectives.
- Host: degree-balanced bin-packing assigns each dst to a (core, tile, slot)
  bin so that every (core, tile, src-half) holds <=1024 edges -> a perfectly
  uniform 8-chunk-per-half layout (minimal gather rows, one SPMD program for
  all 8 cores). Src split in two halves (int16 gather-index limit). Per-dst
  1/deg is precomputed on host and uploaded (no count matmuls).
- Device per core: dma_gather (MoE row-gather ucode) pulls per-edge src rows
  HBM->SBUF in f16; scatter-add via one-hot matmul (f16) on the TensorEngine
  accumulates [dst,128] sums in PSUM; scale by 1/deg; transpose; two linear
  layers + bias via PE matmuls (all f16 operands, f32 PSUM); DMA out the
  [6250,128] f32 shard.
"""

import sys
import types

import numpy as np

N_SRC = 50000
N_DST = 50000
E = 800000
D = 128
OUT = 128
N_CORES = 8
P = 128
DST_PER_CORE = N_DST // N_CORES          # 6250
TILES = (DST_PER_CORE + P - 1) // P      # 49
HALF = 25000                             # int16 index limit split
# SWDGE ring limit: 1024 rows/gather (1920 wedges the device: NRT 101).
MAX_ROWS_PER_GATHER = 1024


def _install_ntff_hook():
    try:
        import antenv
        if "antenv.axon_hooks" in sys.modules:
            return
        mod = types.ModuleType("antenv.axon_hooks")
        _h = [None]
        mod.set_axon_ntff_profile_hook = lambda h: _h.__setitem__(0, h)
        mod.get_axon_ntff_profile_hook = lambda: _h[0]
        sys.modules["antenv.axon_hooks"] = mod
        antenv.axon_hooks = mod
        from trn_agent_boot.trn_boot import _ntff_profile_via_ctypes
        mod.set_axon_ntff_profile_hook(
            _ntff_profile_via_ctypes("/opt/axon/libaxon_pjrt.so"))
    except Exception:
        pass


def _balance_dsts(edge_src, edge_dst):
    """Assign each dst to a (core, tile, slot) bin so that per-(bin, src-half)
    edge counts are balanced (target <=1024 = 8 chunks of 128).

    Returns (dst_core, dst_tile, dst_slot, bins) where bins[c][t] is the
    ordered list of dst ids in that bin.
    """
    n_bins = N_CORES * TILES                     # 392
    lo_deg = np.bincount(edge_dst[edge_src < HALF], minlength=N_DST)
    hi_deg = np.bincount(edge_dst[edge_src >= HALF], minlength=N_DST)
    # bin capacities: last tile of each core holds the 6250-48*128=106 rest
    cap = np.full(n_bins, P, np.int64)
    cap[TILES - 1::TILES] = DST_PER_CORE - (TILES - 1) * P   # 106
    order = np.argsort(-(lo_deg + hi_deg), kind="stable")
    bin_lo = np.zeros(n_bins, np.int64)
    bin_hi = np.zeros(n_bins, np.int64)
    bin_cnt = np.zeros(n_bins, np.int64)
    members = [[] for _ in range(n_bins)]
    full = np.zeros(n_bins, bool)
    lim = 8 * P                                  # 1024-edge half target
    for d in order:
        nlo = bin_lo + lo_deg[d]
        nhi = bin_hi + hi_deg[d]
        load = np.maximum(nlo, nhi).astype(np.float64)
        load += 1e6 * (np.maximum(nlo - lim, 0) + np.maximum(nhi - lim, 0))
        load[full] = np.inf
        b = int(np.argmin(load))
        members[b].append(int(d))
        bin_lo[b] += lo_deg[d]
        bin_hi[b] += hi_deg[d]
        bin_cnt[b] += 1
        if bin_cnt[b] >= cap[b]:
            full[b] = True
    # swap-repair: force every (bin, half) load <= lim so all tiles use
    # exactly 8 chunks per half (uniform program, minimal gather rows)
    for _ in range(1000):
        over_lo = bin_lo > lim
        over_hi = bin_hi > lim
        if not (over_lo.any() or over_hi.any()):
            break
        use_lo = over_lo.any() and (not over_hi.any()
                                    or bin_lo.max() >= bin_hi.max())
        load = bin_lo if use_lo else bin_hi
        degv = lo_deg if use_lo else hi_deg
        b = int(np.argmax(load))
        b2 = int(np.argmin(load))
        mb = members[b]
        m2 = members[b2]
        d = max(mb, key=lambda x: degv[x])
        d2 = min(m2, key=lambda x: degv[x])
        if degv[d] <= degv[d2]:
            break
        mb[mb.index(d)] = d2
        m2[m2.index(d2)] = d
        bin_lo[b] += lo_deg[d2] - lo_deg[d]
        bin_hi[b] += hi_deg[d2] - hi_deg[d]
        bin_lo[b2] += lo_deg[d] - lo_deg[d2]
        bin_hi[b2] += hi_deg[d] - hi_deg[d2]

    dst_core = np.empty(N_DST, np.int64)
    dst_tile = np.empty(N_DST, np.int64)
    dst_slot = np.empty(N_DST, np.int64)
    bins = [[None] * TILES for _ in range(N_CORES)]
    for b in range(n_bins):
        c, t = divmod(b, TILES)
        ids = np.array(members[b], np.int64)
        bins[c][t] = ids
        dst_core[ids] = c
        dst_tile[ids] = t
        dst_slot[ids] = np.arange(len(ids))
    return dst_core, dst_tile, dst_slot, bins


def _prep_core(edge_src, edge_dst, core, dst_core, dst_tile, dst_slot):
    """Per-core edge structure: for each (tile, half) return the edge lists.

    Returns list over 49 tiles of (src_lo, dstl_lo, src_hi, dstl_hi) where
    src_* are int64 source indices (absolute) and dstl_* are slot-in-tile ids.
    """
    m = dst_core[edge_dst] == core
    es = edge_src[m]
    tid = dst_tile[edge_dst[m]]
    dl = dst_slot[edge_dst[m]]
    order = np.argsort(tid, kind="stable")
    es, tid, dl = es[order], tid[order], dl[order]
    bounds = np.searchsorted(tid, np.arange(TILES + 1))
    tiles = []
    for t in range(TILES):
        a, b = bounds[t], bounds[t + 1]
        s, d = es[a:b], dl[a:b]
        is_lo = s < HALF
        tiles.append((s[is_lo], d[is_lo], s[~is_lo] - HALF, d[~is_lo]))
    return tiles


def _pad_chunks(src, dstl, n_chunks):
    """Pad to n_chunks*128 edges; pad idx=0 (valid row), dstl=-1 (no one-hot)."""
    n = n_chunks * P
    s = np.zeros(n, np.int16)
    d = np.full(n, -1.0, np.float32)
    s[:len(src)] = src.astype(np.int16)
    d[:len(dstl)] = dstl.astype(np.float32)
    return s, d


def _wrap_idx(idx_flat):
    """dma_gather wrapped index layout: index j at partition j%16, col j//16,
    replicated across the 8 gpsimd cores (partition groups of 16)."""
    n = len(idx_flat)
    w = idx_flat.reshape(n // 16, 16).T          # [16, n/16]
    return np.tile(w, (8, 1))                    # [128, n/16]


def build_and_run(x_src, x_dst, edge_src, edge_dst, W_neigh, b_neigh,
                  W_self, b_self):
    _install_ntff_hook()
    from concourse import bacc, bass, mybir, tile
    from concourse.bass_utils import run_bass_kernel_spmd

    F32 = mybir.dt.float32
    F16 = mybir.dt.float16
    import os as _os
    use_f16 = _os.environ.get("BSAGE_F32", "0") != "1"
    DTAB = F16 if use_f16 else F32
    np_tab = np.float16 if use_f16 else np.float32

    # ---------- host-side sharding / layout ----------
    dst_core, dst_tile, dst_slot, bins = _balance_dsts(edge_src, edge_dst)
    per_core_tiles = [
        _prep_core(edge_src, edge_dst, c, dst_core, dst_tile, dst_slot)
        for c in range(N_CORES)]
    # dst ids of core c in output-row order
    core_dst_ids = [np.concatenate(bins[c]) for c in range(N_CORES)]

    # per-dst reciprocal degree, tile-major per core: rdeg[c][p, t]
    deg = np.bincount(edge_dst, minlength=N_DST).astype(np.float32)
    rdeg_full = 1.0 / np.maximum(deg, 1.0)
    rdeg_all = np.zeros((N_CORES, P, TILES), np.float32)
    for c in range(N_CORES):
        shard = np.zeros(TILES * P, np.float32)
        for t in range(TILES):
            ids = bins[c][t]
            shard[t * P:t * P + len(ids)] = rdeg_full[ids]
        rdeg_all[c] = shard.reshape(TILES, P).T

    # uniform chunk counts across cores (SPMD: one program, 8 data sets)
    KL = [max(max(1, -(-len(per_core_tiles[c][t][0]) // P))
              for c in range(N_CORES)) for t in range(TILES)]
    KH = [max(max(1, -(-len(per_core_tiles[c][t][2]) // P))
              for c in range(N_CORES)) for t in range(TILES)]
    KE = [KL[t] + KH[t] for t in range(TILES)]
    NCH = sum(KE)                                 # total chunks per core
    KEMAX = max(KE)

    # quad grouping: a few tiles share one g buffer; chunk layout within a
    # quad: [lo(t0)|lo(t1)|...|hi(t0)|hi(t1)|...]
    _sizes = [8] * ((TILES - 9) // 8) + [4, 2, 1, 1, 1]
    _rem = TILES - sum(_sizes)
    _sizes = [8] * (_rem // 8) + ([_rem % 8] if _rem % 8 else []) + _sizes if _rem > 0 else _sizes
    QUADS = []
    _q = 0
    for _s in _sizes:
        QUADS.append(list(range(_q, _q + _s)))
        _q += _s
    assert _q == TILES, (_q, TILES, _sizes)
    # chunk offset of each (tile, half) within its quad buffer
    chunk_off = {}
    quad_chunks = []
    for qi, qts in enumerate(QUADS):
        off = 0
        for t in qts:
            chunk_off[(t, 0)] = off
            off += KL[t]
        for t in qts:
            chunk_off[(t, 1)] = off
            off += KH[t]
        quad_chunks.append(off)
    KQMAX = max(quad_chunks)

    # gather plan: per quad per half, one contiguous chunk span covering the
    # member tiles' chunks, split into <=8-chunk (1024-row) instructions.
    # gathers: (quad, half, chunk_off_in_quad, n_chunks, idx_col_base)
    gathers = []
    idx_cols = 0                                  # int16 columns consumed
    for qi, qts in enumerate(QUADS):
        for half in (0, 1):
            span = sum((KL if half == 0 else KH)[t] for t in qts)
            base = chunk_off[(qts[0], half)]
            k_done = 0
            while k_done < span:
                k = min(span - k_done, MAX_ROWS_PER_GATHER // P)
                gathers.append((qi, half, base + k_done, k, idx_cols))
                idx_cols += k * 8
                k_done += k
    IDXCOLS = idx_cols

    # per-core data arrays
    idx_all = np.zeros((N_CORES, P, IDXCOLS), np.int16)
    dstl_all = np.zeros((N_CORES, P, NCH), np.float16)
    cbase = np.concatenate([[0], np.cumsum(KE)])  # chunk col base per tile
    for c in range(N_CORES):
        for t in range(TILES):
            s_lo, d_lo, s_hi, d_hi = per_core_tiles[c][t]
            sl, dl = _pad_chunks(s_lo, d_lo, KL[t])
            sh, dh = _pad_chunks(s_hi, d_hi, KH[t])
            d_cat = np.concatenate([dl, dh])
            # dstl layout: [128, NCH]; slot p of chunk k = edge k*128+p
            dstl_all[c][:, cbase[t]:cbase[t + 1]] = (
                d_cat.reshape(KE[t], P).T.astype(np.float16))
        # per-quad padded source-index streams (chunk layout order)
        quad_src = []
        for qi, qts in enumerate(QUADS):
            parts = []
            for t in qts:
                s_lo, d_lo, _, _ = per_core_tiles[c][t]
                parts.append(_pad_chunks(s_lo, d_lo, KL[t])[0])
            for t in qts:
                _, _, s_hi, d_hi = per_core_tiles[c][t]
                parts.append(_pad_chunks(s_hi, d_hi, KH[t])[0])
            quad_src.append(np.concatenate(parts))
        for (qi, half, off, k, colb) in gathers:
            rows = quad_src[qi][off * P:(off + k) * P]
            idx_all[c][:, colb:colb + k * 8] = _wrap_idx(rows)

    x_lo = np.ascontiguousarray(x_src[:HALF]).astype(np_tab)
    x_hi = np.ascontiguousarray(x_src[HALF:]).astype(np_tab)
    xdstT = np.zeros((N_CORES, P, TILES * P), np.float16)
    for c in range(N_CORES):
        for t in range(TILES):
            ids = bins[c][t]
            xdstT[c][:, t * P:t * P + len(ids)] = (
                x_dst[ids].T.astype(np.float16))
    iota = np.tile(np.arange(P, dtype=np.float16), (P, 1))
    ident = np.eye(P, dtype=np.float32)
    wn = W_neigh.astype(np.float16)
    ws = W_self.astype(np.float16)
    bsum = (b_neigh + b_self).astype(np.float16)[None, :]  # [1,128]

    # ---------- device program ----------
    nc = bacc.Bacc("TRN2", target_bir_lowering=False, debug=False,
                   num_devices=N_CORES, num_swdge_queues=4)
    xlo_d = nc.dram_tensor("xlo", [HALF, D], DTAB, kind="ExternalInput").ap()
    xhi_d = nc.dram_tensor("xhi", [HALF, D], DTAB, kind="ExternalInput").ap()
    idx_d = nc.dram_tensor("idx", [P, IDXCOLS], mybir.dt.int16,
                           kind="ExternalInput").ap()
    dstl_d = nc.dram_tensor("dstl", [P, NCH], F16, kind="ExternalInput").ap()
    rdeg_d = nc.dram_tensor("rdeg", [P, TILES], F32, kind="ExternalInput").ap()
    xdstT_d = nc.dram_tensor("xdstT", [P, TILES * P], F16,
                             kind="ExternalInput").ap()
    iota_d = nc.dram_tensor("iota", [P, P], F16, kind="ExternalInput").ap()
    ident_d = nc.dram_tensor("ident", [P, P], F32, kind="ExternalInput").ap()
    wn_d = nc.dram_tensor("wn", [D, OUT], F16, kind="ExternalInput").ap()
    ws_d = nc.dram_tensor("ws", [D, OUT], F16, kind="ExternalInput").ap()
    bsum_d = nc.dram_tensor("bsum", [1, OUT], F16, kind="ExternalInput").ap()
    out_d = nc.dram_tensor("out", [DST_PER_CORE, OUT], F16,
                           kind="ExternalOutput").ap()

    with tile.TileContext(nc) as tc:
        with (
            tc.tile_pool(name="const", bufs=1) as cpool,
            tc.tile_pool(name="work", bufs=4) as wpool,
            tc.tile_pool(name="psum", bufs=2, space="PSUM") as ppool,
        ):
            # per-quad idx tiles: the first gather only depends on quad 0's
            # small idx slice, not the whole upload
            qbase = []
            qcols = []
            for qi in range(len(QUADS)):
                cols = [g[4] for g in gathers if g[0] == qi]
                ncols = [g[4] + g[3] * 8 for g in gathers if g[0] == qi]
                qbase.append(min(cols))
                qcols.append(max(ncols) - min(cols))
            idx_sbs = [cpool.tile([P, qcols[qi]], mybir.dt.int16,
                                  name=f"idxq{qi}")
                       for qi in range(len(QUADS))]
            dstl_sb = cpool.tile([P, NCH], F16)
            rdeg_sb = cpool.tile([P, TILES], F32)
            xdstT_sb = cpool.tile([P, TILES * P], F16)
            iota_sb = cpool.tile([P, P], F16)
            wn_sb = cpool.tile([D, OUT], F16)
            ws_sb = cpool.tile([D, OUT], F16)
            bsum_sb = cpool.tile([1, OUT], F16)
            ones_row = cpool.tile([1, P], F16)
            ident_sb = cpool.tile([P, P], F32)
            for qi in range(len(QUADS)):
                nc.sync.dma_start(
                    out=idx_sbs[qi][:],
                    in_=idx_d[:, qbase[qi]:qbase[qi] + qcols[qi]])
            nc.sync.dma_start(out=dstl_sb[:], in_=dstl_d[:])
            nc.sync.dma_start(out=iota_sb[:], in_=iota_d[:])
            nc.scalar.dma_start(out=rdeg_sb[:], in_=rdeg_d[:])
            nc.scalar.dma_start(out=xdstT_sb[:], in_=xdstT_d[:])
            nc.scalar.dma_start(out=wn_sb[:], in_=wn_d[:])
            nc.scalar.dma_start(out=ws_sb[:], in_=ws_d[:])
            nc.scalar.dma_start(out=bsum_sb[:], in_=bsum_d[:])
            nc.scalar.dma_start(out=ident_sb[:], in_=ident_d[:])
            nc.vector.memset(ones_row[:], 1.0)

            def _emit_tile(t, g_sb):
                ke, kl = KE[t], KL[t]
                # batched one-hot, built per src-half so the lo-half matmuls
                # only depend on the lo gather (better pipeline overlap):
                # oh[p, k*128+j] = (iota[p,j] == dstl[p,cb+k])
                oh_sb = wpool.tile([P, KEMAX * P], DTAB, tag="oh", name=f"oh{t}")
                i_ap = iota_sb[:]
                d_ap = dstl_sb[:]
                o_ap = oh_sb[:]
                ps1 = ppool.tile([P, D], F32, tag="ps1", name=f"ps1_{t}",
                                 space="PSUM", bufs=3)
                for half, k0, k1 in ((0, 0, kl), (1, kl, ke)):
                    nk = k1 - k0
                    iota3d = bass.AP(i_ap.tensor, i_ap.offset,
                                     [i_ap.ap[0], [0, nk], [i_ap.ap[1][0], P]])
                    dstl3d = bass.AP(
                        d_ap.tensor, d_ap.offset + int(cbase[t]) + k0,
                        [d_ap.ap[0], [d_ap.ap[1][0], nk], [0, P]])
                    oh3d = bass.AP(o_ap.tensor, o_ap.offset + k0 * P,
                                   [o_ap.ap[0], [P, nk], [1, P]])
                    nc.vector.tensor_tensor(out=oh3d, in0=iota3d, in1=dstl3d,
                                            op=mybir.AluOpType.is_equal)
                    gb = chunk_off[(t, half)] - k0
                    for k in range(k0, k1):
                        gk = gb + k
                        nc.tensor.matmul(
                            out=ps1[:, 0:D],
                            lhsT=oh_sb[:, k * P:(k + 1) * P],
                            rhs=g_sb[:, gk * P:(gk + 1) * P],
                            start=(k == 0), stop=(k == ke - 1))

                # agg = sums * (1/deg): per-partition scale on the scalar eng
                agg_sb = wpool.tile([P, D], F32, tag="agg", name=f"agg{t}")
                nc.scalar.mul(out=agg_sb[:], in_=ps1[:, 0:D],
                              mul=rdeg_sb[:, t:t + 1])
                ps_t = ppool.tile([P, P], F32, tag="pst", name=f"pst{t}",
                                  space="PSUM", bufs=3)
                nc.tensor.transpose(out=ps_t[:], in_=agg_sb[:],
                                    identity=ident_sb[:])
                aggT_sb = wpool.tile([P, D], F16, tag="aggT", name=f"agT{t}")
                nc.vector.tensor_copy(out=aggT_sb[:], in_=ps_t[:])

                ps2 = ppool.tile([P, OUT], F32, tag="ps2", name=f"ps2_{t}",
                                 space="PSUM")
                nc.tensor.matmul(out=ps2[:], lhsT=aggT_sb[:], rhs=wn_sb[:],
                                 start=True, stop=False)
                nc.tensor.matmul(out=ps2[:],
                                 lhsT=xdstT_sb[:, t * P:(t + 1) * P],
                                 rhs=ws_sb[:], start=False, stop=False)
                nc.tensor.matmul(out=ps2[:], lhsT=ones_row[:], rhs=bsum_sb[:],
                                 start=False, stop=True)
                o_sb = wpool.tile([P, OUT], F16, tag="osb", name=f"o{t}")
                nc.scalar.copy(out=o_sb[:], in_=ps2[:])
                rows = min(P, DST_PER_CORE - t * P)
                nc.sync.dma_start(out=out_d[t * P:t * P + rows, :],
                                  in_=o_sb[:rows, :])

            gq = [0]
            g_by_quad = [[] for _ in range(len(QUADS))]
            for g in gathers:
                g_by_quad[g[0]].append(g)
            for qi, qts in enumerate(QUADS):
                g_sb = wpool.tile([P, KQMAX * P], DTAB, tag="g", name=f"g{qi}", bufs=3)
                for (_, half, off, k, colb) in g_by_quad[qi]:
                    t_ap = g_sb[:]
                    out3d = bass.AP(t_ap.tensor, t_ap.offset + off * P,
                                    [t_ap.ap[0], [P, k], [1, P]])
                    lcol = colb - qbase[qi]
                    nc.gpsimd.dma_gather(
                        out3d,
                        (xlo_d if half == 0 else xhi_d)[:],
                        idx_sbs[qi][:, lcol:lcol + k * 8],
                        k * P,
                        k * P,
                        D,
                        queue_num=(gq[0] % 4),
                    )
                    gq[0] += 1
                for t in qts:
                    _emit_tile(t, g_sb)

    nc.finalize()

    in_maps = [{
        "xlo": x_lo, "xhi": x_hi, "idx": idx_all[c], "dstl": dstl_all[c],
        "rdeg": rdeg_all[c], "xdstT": xdstT[c], "iota": iota, "ident": ident,
        "wn": wn, "ws": ws, "bsum": bsum,
    } for c in range(N_CORES)]

    import os
    trace = os.environ.get("BSAGE_TRACE", "0") == "1"
    res = run_bass_kernel_spmd(nc, in_maps, core_ids=list(range(N_CORES)),
                               trace=trace)
    out = np.zeros((N_DST, OUT), np.float32)
    for c in range(N_CORES):
        out[core_dst_ids[c]] = res.results[c]["out"].astype(np.float32)
    if trace:
        build_and_run.last_exec_ns = res.exec_time_ns
    return out


def kernel(x_src, x_dst, edge_src, edge_dst, num_dst, W_neigh, b_neigh,
           W_self, b_self):
    x_src = np.asarray(x_src, dtype=np.float32)
    x_dst = np.asarray(x_dst, dtype=np.float32)
    edge_src = np.asarray(edge_src).astype(np.int64)
    edge_dst = np.asarray(edge_dst).astype(np.int64)
    W_neigh = np.asarray(W_neigh, dtype=np.float32)
    b_neigh = np.asarray(b_neigh, dtype=np.float32)
    W_self = np.asarray(W_self, dtype=np.float32)
    b_self = np.asarray(b_self, dtype=np.float32)
    return build_and_run(x_src, x_dst, edge_src, edge_dst, W_neigh, b_neigh,
                         W_self, b_self)
